# revision 31
# baseline (speedup 1.0000x reference)
"""Bass/Tile TRN2 kernel for nn_BertEncoder_41592463294989.

4-layer BERT encoder, KERPLE attention bias, GLU MLP.
Sharding: data-parallel over batch (B=8 -> 8 cores, 1 sequence each).

Per-core layout: activations transposed [feature, token] so every matmul
contracts over the partition dim and LayerNorm reductions (over features)
are done with ones-vector matmuls on the PE.

v2 design:
 - KERPLE bias is Toeplitz (depends only on |i-j|): exp(bias) is
   precomputed on the HOST per (layer, head) and shipped as a bf16 DRAM
   table; softmax becomes p = exp(s + padmask)*ekb. This removes all
   per-element exp/ln/pow work for the bias on the device (was 2/3 of
   ACT + half of attention DVE time).
 - All matmul operands (weights and activations) are bf16: full PE rate,
   half the weight-DMA bytes, 2x DVE rate on bf16 elementwise ops. The
   residual stream (z, h, ao) stays fp32; bf16 shadow copies feed matmuls.
 - All linear-layer biases are folded into the PE via rank-1 matmuls
   (bias row stationary, ones vector moving) instead of DVE/ACT adds.
 - V weights host-packed into per-head 65-column slots (64 features + a
   ones column) so each PV matmul also produces the softmax denominator.
 - partition broadcasts (1/denominator, LN mu/rstd) via K=1 ones-matmuls.
 - GLU and the wo projection are fused per 128-row chunk.
 - Weights packed into few DRAM tensors (dispatch cost scales with arg
   count in the PJRT path).
"""
import contextlib

import numpy as np
import ml_dtypes

import concourse.bass as bass
from concourse import bacc
import concourse.mybir as mybir
import concourse.tile as tile
from concourse.bass_utils import run_bass_kernel_spmd
from concourse.tile_rust import add_dep_helper

B, S, HID, NH, INTER, L = 8, 512, 768, 12, 3072, 4
DH = HID // NH          # 64
P = 128
NT = S // P             # 4 token tiles
KC = HID // P           # 6 hidden chunks
NIC = INTER // P        # 24 intermediate chunks
F32 = mybir.dt.float32
F32R = mybir.dt.float32r
BF16 = mybir.dt.bfloat16
NPBF16 = ml_dtypes.bfloat16
AF = mybir.ActivationFunctionType
ALU = mybir.AluOpType
HALF = NH * 65 // 2     # 390

_BUILT = {}


def _prefer_combined_act_table(arch):
    """Steer the act-table-load pass to the natural_log_exp set for exp/ln.

    The placement pass greedily first-matches each activation function
    against the table list, so alternating exp/ln picks two different
    tables and reloads on every switch. Removing exp/ln from the
    single-function sets (in the cached dict, canonical indices unchanged)
    makes both resolve to the combined set -> no reloads. The emitted
    act_func_set_id still indexes the canonical act_info.json, and the
    combined table genuinely contains both functions.
    """
    from concourse.hw_specs import get_activation_tables
    tabs = get_activation_tables(arch)
    for nm in list(tabs):
        if nm == "natural_log_exp_and_others":
            continue
        tabs[nm].discard(AF.Exp)
        tabs[nm].discard(AF.Ln)


def _layernorm(nc, tc, z_t, out_t, g_t, b_t, ones_col, ones_row, z2p, smp):
    """LN over the feature (partition x chunk) axis of z_t [P, KC, S] (F32R).

    out_t may be BF16 (mid-stack: matmul operand + residual) or F32R (last).
    """
    EPS = 1e-12
    with tc.tile_pool(name="ln_ps", bufs=1, space="PSUM") as ln_ps, \
         tc.tile_pool(name="lnb_ps", bufs=1, space="PSUM") as lnb_ps:
        ps_sz = ln_ps.tile([1, S], F32, tag="sz")
        ps_sz2 = ln_ps.tile([1, S], F32, tag="sz2")
        for c in range(KC):
            nc.tensor.matmul(ps_sz[:], ones_col[:], z_t[:, c, :],
                             start=(c == 0), stop=(c == KC - 1))
        for c in range(KC):
            z2 = z2p.tile([P, S], F32R, tag="ztmp", name=f"zsq{c}")
            nc.vector.tensor_tensor(z2[:], z_t[:, c, :].bitcast(F32),
                                    z_t[:, c, :].bitcast(F32), ALU.mult)
            nc.tensor.matmul(ps_sz2[:], ones_col[:], z2[:],
                             start=(c == 0), stop=(c == KC - 1))
        mu = smp.tile([1, S], F32R, tag="sm", name="mu")
        nc.vector.tensor_scalar(mu[:], ps_sz[:], 1.0 / HID, None, ALU.mult)
        m2 = smp.tile([1, S], F32, tag="sm", name="m2")
        nc.vector.tensor_scalar(m2[:], ps_sz2[:], 1.0 / HID, EPS, ALU.mult, ALU.add)
        musq = smp.tile([1, S], F32, tag="sm", name="musq")
        nc.scalar.activation(musq[:], mu[:].bitcast(F32), AF.Square,
                             bias=0.0, scale=1.0)
        var = smp.tile([1, S], F32, tag="sm", name="var")
        nc.vector.tensor_tensor(var[:], m2[:], musq[:], ALU.subtract)
        lnv = smp.tile([1, S], F32, tag="sm", name="lnv")
        nc.scalar.activation(lnv[:], var[:], AF.Ln, bias=0.0, scale=1.0)
        rstd = smp.tile([1, S], F32R, tag="sm", name="rstd")
        nc.scalar.activation(rstd[:], lnv[:], AF.Exp, bias=0.0, scale=-0.5)
        ps_mu = lnb_ps.tile([P, S], F32, tag="mub")
        nc.tensor.matmul(ps_mu[:], ones_row[:], mu[:], start=True, stop=True)
        ps_rs = lnb_ps.tile([P, S], F32, tag="rsb")
        nc.tensor.matmul(ps_rs[:], ones_row[:], rstd[:], start=True, stop=True)
        for c in range(KC):
            t1 = z2p.tile([P, S], F32, tag="ztmp", name=f"lnt{c}")
            nc.vector.tensor_tensor(t1[:], z_t[:, c, :].bitcast(F32), ps_mu[:],
                                    ALU.subtract)
            nc.vector.tensor_tensor(t1[:], t1[:], ps_rs[:], ALU.mult)
            nc.vector.tensor_scalar(out_t[:, c, :], t1[:], g_t[:, c:c + 1],
                                    b_t[:, c:c + 1], ALU.mult, ALU.add)


def _build(n_layers: int):
    nc = bacc.Bacc("TRN2", target_bir_lowering=False)
    try:
        _prefer_combined_act_table(nc.m.arch)
    except Exception:
        pass

    def inp(name, shape, dt=F32):
        return nc.declare_dram_parameter(name, list(shape), dt, isOutput=False)

    # fp32 consts: hT | mb | maskb | ones_row | ones_col | ln params
    hT_d = inp("hT", [HID, S])
    c32_d = inp("c32", [P, NT + S + 2 + 4 * L * KC])
    ones_row_d = inp("ones_row", [1, P])
    ones_col_d = inp("ones_col", [P, 1])
    # bf16: big weight blob, per-layer layout (offsets in elements):
    #   wqk [2KC, P, KC, P] | wva [2, P, KC, HALF] | woa [KC, P, KC, P]
    #   glu [NIC, P, KC, 256] | wot [INTER, HID]
    #   bqk [2KC*P] | bva [NH*65] | boa [KC*P] | bwo [KC*P]
    W_QKV = 2 * KC * P * KC * P
    W_V = 2 * P * KC * HALF
    W_OA = KC * P * KC * P
    W_GLU = NIC * P * KC * 256
    W_OT = INTER * HID
    W_B = 2 * KC * P + NH * 65 + KC * P + KC * P
    WLAY = W_QKV + W_V + W_OA + W_GLU + W_OT + W_B
    w16_d = inp("w16", [L * WLAY], BF16)
    h16_d = inp("h16", [HID, S], BF16)
    ones_s_d = inp("ones_s", [1, S], BF16)
    ekb_d = inp("ekb", [L, NH, P, NT * S], BF16)
    out_d = nc.declare_dram_parameter("out", [HID, S], F32, isOutput=True)

    def wslice(l, off, sz, shape):
        base = l * WLAY + off
        pat = " ".join(f"d{i}" for i in range(len(shape)))
        return w16_d[base:base + sz].rearrange(
            f"({pat}) -> {pat}", **{f"d{i}": shape[i] for i in range(len(shape))})

    O_QKV = 0
    O_V = O_QKV + W_QKV
    O_OA = O_V + W_V
    O_GLU = O_OA + W_OA
    O_OT = O_GLU + W_GLU
    O_BQK = O_OT + W_OT
    O_BVA = O_BQK + 2 * KC * P
    O_BOA = O_BVA + NH * 65
    O_BWO = O_BOA + KC * P

    with tile.TileContext(nc) as tc:
        lp = nc.allow_low_precision(reason="bf16 matmul operands; loose tol")
        lp.__enter__()
        stack = contextlib.ExitStack()
        const = stack.enter_context(tc.tile_pool(name="const", bufs=1))
        hpool = stack.enter_context(tc.tile_pool(name="hpool", bufs=2))
        h16p = stack.enter_context(tc.tile_pool(name="h16p", bufs=2))
        qkp = stack.enter_context(tc.tile_pool(name="qkp", bufs=1))
        vap = stack.enter_context(tc.tile_pool(name="vap", bufs=1))
        p4p = stack.enter_context(tc.tile_pool(name="p4p", bufs=2))
        ekbp = stack.enter_context(tc.tile_pool(name="ekbp", bufs=3))
        up = stack.enter_context(tc.tile_pool(name="up", bufs=2))
        atp = stack.enter_context(tc.tile_pool(name="atp", bufs=1))
        smp = stack.enter_context(tc.tile_pool(name="smp", bufs=3))
        zp = stack.enter_context(tc.tile_pool(name="zp", bufs=1))
        z2p = stack.enter_context(tc.tile_pool(name="z2p", bufs=2))
        aop = stack.enter_context(tc.tile_pool(name="aop", bufs=1))
        ao16p = stack.enter_context(tc.tile_pool(name="ao16p", bufs=1))
        xcp = stack.enter_context(tc.tile_pool(name="xcp", bufs=2))
        xgp = stack.enter_context(tc.tile_pool(name="xgp", bufs=2))
        wst = stack.enter_context(tc.tile_pool(name="wst", bufs=3))   # [128,KC,128] stream
        wvp = stack.enter_context(tc.tile_pool(name="wvp", bufs=2))   # wva halves
        wgp = stack.enter_context(tc.tile_pool(name="wgp", bufs=3))   # glu [128,KC,256]
        wop = stack.enter_context(tc.tile_pool(name="wop", bufs=3))   # wot [128,768]
        bp = stack.enter_context(tc.tile_pool(name="bp", bufs=2))
        bvp = stack.enter_context(tc.tile_pool(name="bvp", bufs=1))

        # ---- constants ----
        c32_t = const.tile([P, NT + S + 2 + 4 * L * KC], F32)
        nc.sync.dma_start(c32_t[:], c32_d[:])
        mb_t = c32_t[:, 0:NT]
        maskb_t = c32_t[:, NT:NT + S]
        lnp_t = c32_t[:, NT + S + 2:]   # [P, 4*L*KC]: l1g|l1b|l2g|l2b per layer
        ones_row = const.tile([1, P], F32R)
        nc.sync.dma_start(ones_row[:], ones_row_d[:].bitcast(F32R))
        ones_col = const.tile([P, 1], F32R)
        nc.sync.dma_start(ones_col[:], ones_col_d[:].bitcast(F32R))
        ones_s = const.tile([1, S], BF16)
        nc.sync.dma_start(ones_s[:], ones_s_d[:])

        # layer 0 hidden state (bf16: matmul operand + residual stream)
        h16_t = h16p.tile([P, KC, S], BF16, tag="h16")
        nc.sync.dma_start(h16_t[:], h16_d[:].rearrange("(c p) t -> p c t", p=P))

        last_gelu = [None]
        prev_exp = [None]
        for l in range(n_layers):
            ln1g_t = lnp_t[:, (4 * l) * KC:(4 * l + 1) * KC]
            ln1b_t = lnp_t[:, (4 * l + 1) * KC:(4 * l + 2) * KC]
            ln2g_t = lnp_t[:, (4 * l + 2) * KC:(4 * l + 3) * KC]
            ln2b_t = lnp_t[:, (4 * l + 3) * KC:(4 * l + 4) * KC]

            with tc.tile_pool(name="qkv_ps", bufs=2, space="PSUM") as qkv_ps, \
                 tc.tile_pool(name="sc_ps", bufs=4, space="PSUM") as sc_ps, \
                 tc.tile_pool(name="pv_ps", bufs=2, space="PSUM") as pv_ps:
                # ---------- V (token-major, head-slotted + ones col) ----------
                bva_t = bvp.tile([1, NH * 65], BF16, tag="bva")
                nc.sync.dma_start(bva_t[:], wslice(l, O_BVA, NH * 65, (1, NH * 65)))
                bqk_t = bp.tile([1, 2 * KC, P], BF16, tag="bqk")
                nc.sync.dma_start(bqk_t[:],
                                  wslice(l, O_BQK, 2 * KC * P, (1, 2 * KC, P)))
                va_t = vap.tile([P, NT, NH * 65], BF16, tag="va")
                for half in range(2):
                    sl = slice(half * HALF, (half + 1) * HALF)
                    wv = wvp.tile([P, KC, HALF], BF16, tag="wv", name=f"wv{half}")
                    nc.sync.dma_start(
                        wv[:], wslice(l, O_V + half * P * KC * HALF, P * KC * HALF,
                                      (P, KC, HALF)))
                    for jt in range(NT):
                        ps = qkv_ps.tile([P, HALF], F32, tag="qkvps",
                                         name=f"vps{half}_{jt}")
                        for kc in range(KC):
                            nc.tensor.matmul(ps[:], h16_t[:, kc, jt * P:(jt + 1) * P],
                                             wv[:, kc, :], start=(kc == 0), stop=False)
                        nc.tensor.matmul(ps[:], ones_s[:, 0:P],
                                         bva_t[:, sl], start=False, stop=True)
                        nc.scalar.activation(va_t[:, jt, sl], ps[:], AF.Copy)

                # ---------- QK + attention, interleaved per head pair ----------
                qk_t = qkp.tile([P, 2 * KC, S], BF16, tag="qk")
                at16 = atp.tile([P, KC, S], BF16, tag="attnT")

                def flush_tail(pend):
                    """Denominator tail of a finished head (deferred one head
                    so the bc matmul's wait on rec doesn't head-of-line-block
                    the next head's score matmuls in the PE FIFO)."""
                    ps_pv, h = pend
                    rec = smp.tile([1, S], F32R, tag="sm", name="rec")
                    nc.vector.reciprocal(rec[:], ps_pv[64:65, :])
                    ps_bc = qkv_ps.tile([64, S], F32, tag="qkvps", name="bc")
                    nc.tensor.matmul(ps_bc[:], ones_row[:, 0:64], rec[:],
                                     start=True, stop=True)
                    rb_sb = up.tile([64, S], F32, tag="rb", name="rb_sb")
                    nc.scalar.activation(rb_sb[:], ps_bc[:], AF.Copy)
                    nc.vector.tensor_tensor(
                        at16[64 * (h % 2):64 * (h % 2) + 64, h // 2, :],
                        ps_pv[0:64, :], rb_sb[:], ALU.mult)

                pending = None
                for hp in range(KC):
                    for ot in (KC + hp, hp):     # k chunk, then q chunk
                        ps = qkv_ps.tile([P, S], F32, tag="qkvps",
                                         name=f"qk{ot}")
                        w = wst.tile([P, KC, P], BF16, tag="w", name=f"wqk{ot}")
                        nc.sync.dma_start(
                            w[:], wslice(l, O_QKV + ot * P * KC * P, P * KC * P,
                                         (P, KC, P)))
                        for kc in range(KC):
                            nc.tensor.matmul(ps[:], w[:, kc, :], h16_t[:, kc, :],
                                             start=(kc == 0), stop=False)
                        nc.tensor.matmul(ps[:], bqk_t[:, ot, :], ones_s[:],
                                         start=False, stop=True)
                        nc.scalar.activation(qk_t[:, ot, :], ps[:], AF.Copy)
                    for h in (2 * hp, 2 * hp + 1):
                        koff = (DH * h) % P
                        qoff = (DH * h) % P
                        ekb_t = ekbp.tile([P, NT * S], BF16, tag="ekb",
                                          name=f"ekb{h}")
                        nc.sync.dma_start(ekb_t[:], ekb_d[l, h])
                        p4 = p4p.tile([P, NT, S], BF16, tag="p4")
                        ps_pv = pv_ps.tile([65, S], F32, tag="pv")
                        ps_ss = []
                        for jt in range(NT):
                            ps_s = sc_ps.tile([P, S], F32, tag="sc", name=f"sc{jt}")
                            ps_ss.append(ps_s)
                            nc.tensor.matmul(
                                ps_s[:],
                                qk_t[koff:koff + DH, KC + hp, jt * P:(jt + 1) * P],
                                qk_t[qoff:qoff + DH, hp, :],
                                start=True, stop=True)
                        if pending is not None:
                            flush_tail(pending)
                        for jt in range(NT):
                            _i = nc.scalar.activation(p4[:, jt, :], ps_ss[jt][:],
                                                      AF.Exp,
                                                      bias=mb_t[:, jt:jt + 1],
                                                      scale=1.0)
                            if h == 0 and jt == 0 and last_gelu[0] is not None:
                                add_dep_helper(_i.ins, last_gelu[0].ins, False,
                                               "act table grouping")
                            prev_exp[0] = _i
                            nc.vector.tensor_tensor(
                                p4[:, jt, :], p4[:, jt, :],
                                ekb_t[:, jt * S:(jt + 1) * S], ALU.mult)
                            nc.tensor.matmul(ps_pv[:],
                                             va_t[:, jt, 65 * h:65 * h + 65],
                                             p4[:, jt, :], start=(jt == 0),
                                             stop=(jt == NT - 1))
                        pending = (ps_pv, h)
                flush_tail(pending)
                pending = None

                # ---------- attention out projection + residual ----------
                boa_t = bp.tile([1, KC, P], BF16, tag="boa")
                nc.sync.dma_start(boa_t[:], wslice(l, O_BOA, KC * P, (1, KC, P)))
                z_t = zp.tile([P, KC, S], F32R, tag="z")
                for ot in range(KC):
                    ps = sc_ps.tile([P, S], F32, tag="sc", name=f"prj{ot}")
                    w = wst.tile([P, KC, P], BF16, tag="w", name=f"woa{ot}")
                    nc.sync.dma_start(
                        w[:], wslice(l, O_OA + ot * P * KC * P, P * KC * P,
                                     (P, KC, P)))
                    for kc in range(KC):
                        nc.tensor.matmul(ps[:], w[:, kc, :], at16[:, kc, :],
                                         start=(kc == 0), stop=False)
                    nc.tensor.matmul(ps[:], boa_t[:, ot, :], ones_s[:],
                                     start=False, stop=True)
                    nc.vector.tensor_tensor(z_t[:, ot, :], ps[:],
                                            h16_t[:, ot, :], ALU.add)

            # ---------- LN1 ----------
            ao16 = ao16p.tile([P, KC, S], BF16, tag="ao16")
            _layernorm(nc, tc, z_t, ao16, ln1g_t, ln1b_t, ones_col,
                       ones_row, z2p, smp)

            # ---------- GLU + wo (fused) ----------
            with tc.tile_pool(name="glu_ps", bufs=1, space="PSUM") as glu_ps, \
                 tc.tile_pool(name="wo_ps", bufs=6, space="PSUM") as wo_ps:
                bwo_t = bp.tile([1, KC, P], BF16, tag="bwo")
                nc.sync.dma_start(bwo_t[:], wslice(l, O_BWO, KC * P, (1, KC, P)))

                wo_acc = [wo_ps.tile([P, S], F32, tag="woacc", name=f"woacc{i}")
                          for i in range(KC)]
                for gt in range(NIC):
                    ps_g = glu_ps.tile([P, S], F32, tag="gps")
                    ps_u = glu_ps.tile([P, S], F32, tag="ups")
                    gw = wgp.tile([P, KC, 256], BF16, tag="gw", name=f"gw{gt}")
                    nc.sync.dma_start(
                        gw[:], wslice(l, O_GLU + gt * P * KC * 256, P * KC * 256,
                                      (P, KC, 256)))
                    for kc in range(KC):
                        nc.tensor.matmul(ps_g[:], gw[:, kc, 0:128], ao16[:, kc, :],
                                         start=(kc == 0), stop=(kc == KC - 1))
                    for kc in range(KC):
                        nc.tensor.matmul(ps_u[:], gw[:, kc, 128:256], ao16[:, kc, :],
                                         start=(kc == 0), stop=(kc == KC - 1))
                    xg = xgp.tile([P, S], BF16, tag="xg")
                    _i = nc.scalar.activation(xg[:], ps_g[:], AF.Gelu)
                    if gt == 0 and prev_exp[0] is not None:
                        add_dep_helper(_i.ins, prev_exp[0].ins, False,
                                       "act table grouping")
                    last_gelu[0] = _i
                    xc = xcp.tile([P, S], BF16, tag="xc")
                    nc.vector.tensor_tensor(xc[:], xg[:], ps_u[:], ALU.mult)
                    wot_t = wop.tile([P, HID], BF16, tag="wot")
                    nc.sync.dma_start(
                        wot_t[:], wslice(l, O_OT + gt * P * HID, P * HID, (P, HID)))
                    for ot in range(KC):
                        nc.tensor.matmul(wo_acc[ot][:], wot_t[:, ot * P:(ot + 1) * P],
                                         xc[:], start=(gt == 0), stop=False)
                z2_t = zp.tile([P, KC, S], F32R, tag="z", name="z_mlp")
                for ot in range(KC):
                    nc.tensor.matmul(wo_acc[ot][:], bwo_t[:, ot, :], ones_s[:],
                                     start=False, stop=True)
                    nc.vector.tensor_tensor(z2_t[:, ot, :], wo_acc[ot][:],
                                            ao16[:, ot, :], ALU.add)

            # ---------- LN2 -> next h ----------
            if l + 1 < n_layers:
                h16_t = h16p.tile([P, KC, S], BF16, tag="h16",
                                  name=f"h16_{l + 1}")
                _layernorm(nc, tc, z2_t, h16_t, ln2g_t, ln2b_t, ones_col,
                           ones_row, z2p, smp)
            else:
                h_t = hpool.tile([P, KC, S], F32R, tag="h", name="h_last")
                _layernorm(nc, tc, z2_t, h_t, ln2g_t, ln2b_t, ones_col,
                           ones_row, z2p, smp)

        # ---------- final mask + store ----------
        out_sb = zp.tile([P, KC, S], F32, tag="z", name="out_sb")
        if n_layers == 0:
            h_t = hpool.tile([P, KC, S], F32R, tag="h", name="h_last")
            nc.sync.dma_start(h_t[:], hT_d[:].rearrange("(c p) t -> p c t",
                                                        p=P).bitcast(F32R))
        for c in range(KC):
            nc.vector.tensor_tensor(out_sb[:, c, :], h_t[:, c, :].bitcast(F32),
                                    maskb_t[:], ALU.mult)
        nc.sync.dma_start(out_d[:].rearrange("(c p) t -> p c t", p=P), out_sb[:])

        stack.close()
        lp.__exit__(None, None, None)

    nc.finalize()
    return nc


def _prep_inputs(hidden_states, attention_mask, Wqkv_w, Wqkv_b, attn_out_w,
                 attn_out_b, ln1_g, ln1_b, glu_w, wo_w, wo_b, ln2_g, ln2_b,
                 r1, r2, r3):
    """Host-side sharding + weight layout transforms (shared across cores)."""
    f32 = np.float32
    shared = {}
    shared["ones_row"] = np.ones((1, P), f32)
    shared["ones_col"] = np.ones((P, 1), f32)
    shared["ones_s"] = np.ones((1, S), NPBF16)

    # ekb: exp(kerple bias) per (layer, head), Toeplitz [S, S] -> [P, NT*S]
    c1 = np.clip(r1.reshape(L, NH).astype(np.float64), 1e-7, None)
    c2 = np.clip(r2.reshape(L, NH).astype(np.float64), 1e-7, None)
    c3 = np.clip(r3.reshape(L, NH).astype(np.float64), 1e-7, None)
    idx = np.arange(S)
    rel = np.abs(idx[None, :] - idx[:, None]).astype(np.float64)  # [j, i]
    ekb = np.empty((L, NH, P, NT * S), NPBF16)
    for l in range(L):
        for h in range(NH):
            relp = np.where(rel > 0, rel, 1.0) ** c3[l, h]
            relp = np.where(rel > 0, relp, 0.0)
            m = np.exp(-c1[l, h] * np.log1p(c2[l, h] * relp))  # [j, i]
            # [j, i] -> [jt, p, i] -> [p, jt, i] -> [p, jt*i]
            ekb[l, h] = np.ascontiguousarray(
                m.reshape(NT, P, S).transpose(1, 0, 2).reshape(P, NT * S)
            ).astype(NPBF16)
    shared["ekb"] = ekb

    wq = Wqkv_w[:, :HID, :] / 8.0           # fold 1/sqrt(DH)
    wk = Wqkv_w[:, HID:2 * HID, :]
    bq = Wqkv_b[:, :HID] / 8.0
    bk = Wqkv_b[:, HID:2 * HID]
    wqk = np.concatenate([wq, wk], axis=1)  # [L, 1536, HID]
    wqkT = np.transpose(wqk, (0, 2, 1))     # [L, HID, 1536]
    wqk_p = np.ascontiguousarray(
        wqkT.reshape(L, KC, P, 2 * KC, P).transpose(0, 3, 2, 1, 4))
    bqk_p = np.concatenate([bq, bk], axis=1)  # [L, 1536]

    wv = Wqkv_w[:, 2 * HID:, :]             # [L, 768v, 768]
    bv = Wqkv_b[:, 2 * HID:]
    wva = np.zeros((L, HID, NH * 65), f32)
    bva_p = np.zeros((L, NH * 65), f32)
    for h in range(NH):
        wva[:, :, 65 * h:65 * h + 64] = np.transpose(
            wv[:, DH * h:DH * (h + 1), :], (0, 2, 1))
        bva_p[:, 65 * h:65 * h + 64] = bv[:, DH * h:DH * (h + 1)]
        bva_p[:, 65 * h + 64] = 1.0
    wva_p = np.ascontiguousarray(
        wva.reshape(L, KC, P, 2, HALF).transpose(0, 3, 2, 1, 4))

    woaT = np.transpose(attn_out_w, (0, 2, 1))  # [L, HID, HID]
    woa_p = np.ascontiguousarray(
        woaT.reshape(L, KC, P, KC, P).transpose(0, 3, 2, 1, 4))

    glu = np.empty((L, HID, NIC, 256), f32)
    gw = np.transpose(glu_w, (0, 2, 1))     # [L, HID, 6144]
    for gt in range(NIC):
        glu[:, :, gt, 0:128] = gw[:, :, gt * P:(gt + 1) * P]
        glu[:, :, gt, 128:256] = gw[:, :, INTER + gt * P:INTER + (gt + 1) * P]
    glu_p = np.ascontiguousarray(
        glu.reshape(L, KC, P, NIC, 256).transpose(0, 3, 2, 1, 4))
    wot_p = np.ascontiguousarray(np.transpose(wo_w, (0, 2, 1)))  # [L, INTER, HID]

    w16 = np.concatenate([
        wqk_p.reshape(L, -1), wva_p.reshape(L, -1), woa_p.reshape(L, -1),
        glu_p.reshape(L, -1), wot_p.reshape(L, -1),
        bqk_p.reshape(L, -1), bva_p.reshape(L, -1),
        attn_out_b.reshape(L, -1), wo_b.reshape(L, -1),
    ], axis=1).astype(NPBF16)
    shared["w16"] = np.ascontiguousarray(w16.reshape(-1))

    def pcol(v):  # [L, 768] -> [L, P, KC]
        return np.ascontiguousarray(v.reshape(L, KC, P).transpose(0, 2, 1)).astype(f32)

    lnp = np.stack([pcol(ln1_g), pcol(ln1_b), pcol(ln2_g), pcol(ln2_b)],
                   axis=1)  # [L, 4, P, KC]
    lnp = np.ascontiguousarray(lnp.transpose(2, 0, 1, 3)).reshape(P, 4 * L * KC)

    in_maps = []
    for b in range(B):
        m = dict(shared)
        hTb = np.ascontiguousarray(hidden_states[b].T).astype(f32)
        m["hT"] = hTb
        m["h16"] = hTb.astype(NPBF16)
        mask = attention_mask[b].astype(f32)          # [S]
        mbias = (1.0 - mask) * -10000.0
        c32 = np.zeros((P, NT + S + 2 + 4 * L * KC), f32)
        c32[:, 0:NT] = mbias.reshape(NT, P).T
        c32[:, NT:NT + S] = mask[None, :]
        c32[:, NT + S + 2:] = lnp
        m["c32"] = c32
        in_maps.append(m)
    return in_maps


def kernel(**inputs) -> np.ndarray:
    n_layers = int(inputs.pop("_n_layers", L))
    if n_layers not in _BUILT:
        _BUILT[n_layers] = _build(n_layers)
    nc = _BUILT[n_layers]
    in_maps = _prep_inputs(**inputs)
    res = run_bass_kernel_spmd(nc, in_maps, list(range(B))).results
    out = np.empty((B, S, HID), np.float32)
    for b in range(B):
        out[b] = res[b]["out"].T
    return out


# revision 35
# speedup vs baseline: 4.0505x; 4.0505x over previous
"""Bass/Tile TRN2 kernel for nn_BertEncoder_41592463294989.

4-layer BERT encoder, KERPLE attention bias, GLU MLP.
Sharding: data-parallel over batch (B=8 -> 8 cores, 1 sequence each).

Per-core layout: activations transposed [feature, token] so every matmul
contracts over the partition dim and LayerNorm reductions (over features)
are done with ones-vector matmuls on the PE.

v2 design:
 - KERPLE bias is Toeplitz (depends only on |i-j|): exp(bias) is
   precomputed on the HOST per (layer, head) and shipped as a bf16 DRAM
   table; softmax becomes p = exp(s + padmask)*ekb. This removes all
   per-element exp/ln/pow work for the bias on the device (was 2/3 of
   ACT + half of attention DVE time).
 - All matmul operands (weights and activations) are bf16: full PE rate,
   half the weight-DMA bytes, 2x DVE rate on bf16 elementwise ops. The
   residual stream (z, h, ao) stays fp32; bf16 shadow copies feed matmuls.
 - All linear-layer biases are folded into the PE via rank-1 matmuls
   (bias row stationary, ones vector moving) instead of DVE/ACT adds.
 - V weights host-packed into per-head 65-column slots (64 features + a
   ones column) so each PV matmul also produces the softmax denominator.
 - partition broadcasts (1/denominator, LN mu/rstd) via K=1 ones-matmuls.
 - GLU and the wo projection are fused per 128-row chunk.
 - Weights packed into few DRAM tensors (dispatch cost scales with arg
   count in the PJRT path).
"""
import contextlib

import numpy as np
import ml_dtypes

import concourse.bass as bass
from concourse import bacc
import concourse.mybir as mybir
import concourse.tile as tile
from concourse.bass_utils import run_bass_kernel_spmd
from concourse.tile_rust import add_dep_helper

B, S, HID, NH, INTER, L = 8, 512, 768, 12, 3072, 4
DH = HID // NH          # 64
P = 128
NT = S // P             # 4 token tiles
KC = HID // P           # 6 hidden chunks
NIC = INTER // P        # 24 intermediate chunks
F32 = mybir.dt.float32
F32R = mybir.dt.float32r
BF16 = mybir.dt.bfloat16
NPBF16 = ml_dtypes.bfloat16
AF = mybir.ActivationFunctionType
ALU = mybir.AluOpType
HALF = NH * 65 // 2     # 390

_BUILT = {}


def _prefer_combined_act_table(arch):
    """Steer the act-table-load pass to the natural_log_exp set for exp/ln.

    The placement pass greedily first-matches each activation function
    against the table list, so alternating exp/ln picks two different
    tables and reloads on every switch. Removing exp/ln from the
    single-function sets (in the cached dict, canonical indices unchanged)
    makes both resolve to the combined set -> no reloads. The emitted
    act_func_set_id still indexes the canonical act_info.json, and the
    combined table genuinely contains both functions.
    """
    from concourse.hw_specs import get_activation_tables
    tabs = get_activation_tables(arch)
    for nm in list(tabs):
        if nm == "natural_log_exp_and_others":
            continue
        tabs[nm].discard(AF.Exp)
        tabs[nm].discard(AF.Ln)


def _layernorm(nc, tc, z_t, out_t, g_t, b_t, ones_col, ones_row, z2p, smp,
               ones_s=None, act_dep=None):
    """LN over the feature (partition x chunk) axis of z_t [P, KC, S] (F32R).

    out_t may be BF16 (mid-stack: matmul operand + residual) or F32R (last).
    If act_dep is given, a tiny dummy Exp is issued first (ordered after
    act_dep) so the natural_log_exp table load happens off the critical path.
    """
    EPS = 1e-12
    with tc.tile_pool(name="ln_ps", bufs=1, space="PSUM") as ln_ps, \
         tc.tile_pool(name="lnb_ps", bufs=1, space="PSUM") as lnb_ps:
        if act_dep is not None:
            dummy = smp.tile([1, 8], F32, tag="dummy", name="tabswitch")
            _d = nc.scalar.activation(dummy[:], ones_s[:, 0:8], AF.Exp,
                                      bias=0.0, scale=1.0)
            add_dep_helper(_d.ins, act_dep.ins, False, "act table prefetch")
        ps_sz = ln_ps.tile([1, S], F32, tag="sz")
        ps_sz2 = ln_ps.tile([1, S], F32, tag="sz2")
        for c in range(KC):
            nc.tensor.matmul(ps_sz[:], ones_col[:], z_t[:, c, :],
                             start=(c == 0), stop=(c == KC - 1))
        for c in range(KC):
            z2 = z2p.tile([P, S], F32R, tag="ztmp", name=f"zsq{c}")
            nc.scalar.activation(z2[:], z_t[:, c, :].bitcast(F32), AF.Square,
                                 bias=0.0, scale=1.0)
            nc.tensor.matmul(ps_sz2[:], ones_col[:], z2[:],
                             start=(c == 0), stop=(c == KC - 1))
        mu = smp.tile([1, S], F32R, tag="sm", name="mu")
        nc.vector.tensor_scalar(mu[:], ps_sz[:], 1.0 / HID, None, ALU.mult)
        m2 = smp.tile([1, S], F32, tag="sm", name="m2")
        nc.vector.tensor_scalar(m2[:], ps_sz2[:], 1.0 / HID, EPS, ALU.mult, ALU.add)
        musq = smp.tile([1, S], F32, tag="sm", name="musq")
        nc.scalar.activation(musq[:], mu[:].bitcast(F32), AF.Square,
                             bias=0.0, scale=1.0)
        var = smp.tile([1, S], F32, tag="sm", name="var")
        nc.vector.tensor_tensor(var[:], m2[:], musq[:], ALU.subtract)
        lnv = smp.tile([1, S], F32, tag="sm", name="lnv")
        nc.scalar.activation(lnv[:], var[:], AF.Ln, bias=0.0, scale=1.0)
        rstd = smp.tile([1, S], F32R, tag="sm", name="rstd")
        nc.scalar.activation(rstd[:], lnv[:], AF.Exp, bias=0.0, scale=-0.5)
        ps_mu = lnb_ps.tile([P, S], F32, tag="mub")
        nc.tensor.matmul(ps_mu[:], ones_row[:], mu[:], start=True, stop=True)
        ps_rs = lnb_ps.tile([P, S], F32, tag="rsb")
        nc.tensor.matmul(ps_rs[:], ones_row[:], rstd[:], start=True, stop=True)
        for c in range(KC):
            t1 = z2p.tile([P, S], F32, tag="ztmp", name=f"lnt{c}")
            nc.vector.tensor_tensor(t1[:], z_t[:, c, :].bitcast(F32), ps_mu[:],
                                    ALU.subtract)
            nc.vector.tensor_tensor(t1[:], t1[:], ps_rs[:], ALU.mult)
            nc.vector.tensor_scalar(out_t[:, c, :], t1[:], g_t[:, c:c + 1],
                                    b_t[:, c:c + 1], ALU.mult, ALU.add)


def _build(n_layers: int):
    nc = bacc.Bacc("TRN2", target_bir_lowering=False)
    try:
        _prefer_combined_act_table(nc.m.arch)
    except Exception:
        pass

    def inp(name, shape, dt=F32):
        return nc.declare_dram_parameter(name, list(shape), dt, isOutput=False)

    # fp32 consts: hT | mb | maskb | ones_row | ones_col | ln params
    hT_d = inp("hT", [HID, S])
    c32_d = inp("c32", [P, NT + S + 2 + 4 * L * KC])
    ones_row_d = inp("ones_row", [1, P])
    ones_col_d = inp("ones_col", [P, 1])
    # bf16: big weight blob, per-layer layout (offsets in elements):
    #   wqk [2KC, P, KC, P] | wva [2, P, KC, HALF] | woa [KC, P, KC, P]
    #   glu [NIC, P, KC, 256] | wot [INTER, HID]
    #   bqk [2KC*P] | bva [NH*65] | boa [KC*P] | bwo [KC*P]
    W_QKV = 2 * KC * P * KC * P
    W_V = 2 * P * KC * HALF
    W_OA = KC * P * KC * P
    W_GLU = NIC * P * KC * 256
    W_OT = INTER * HID
    W_B = 2 * KC * P + NH * 65 + KC * P + KC * P
    WLAY = W_QKV + W_V + W_OA + W_GLU + W_OT + W_B
    w16_d = inp("w16", [L * WLAY], BF16)
    h16_d = inp("h16", [HID, S], BF16)
    ones_s_d = inp("ones_s", [1, S], BF16)
    ekb_d = inp("ekb", [L, NH, P, NT * S], BF16)
    out_d = nc.declare_dram_parameter("out", [HID, S], F32, isOutput=True)

    def wslice(l, off, sz, shape):
        base = l * WLAY + off
        pat = " ".join(f"d{i}" for i in range(len(shape)))
        return w16_d[base:base + sz].rearrange(
            f"({pat}) -> {pat}", **{f"d{i}": shape[i] for i in range(len(shape))})

    O_QKV = 0
    O_V = O_QKV + W_QKV
    O_OA = O_V + W_V
    O_GLU = O_OA + W_OA
    O_OT = O_GLU + W_GLU
    O_BQK = O_OT + W_OT
    O_BVA = O_BQK + 2 * KC * P
    O_BOA = O_BVA + NH * 65
    O_BWO = O_BOA + KC * P

    with tile.TileContext(nc) as tc:
        lp = nc.allow_low_precision(reason="bf16 matmul operands; loose tol")
        lp.__enter__()
        stack = contextlib.ExitStack()
        const = stack.enter_context(tc.tile_pool(name="const", bufs=1))
        hpool = stack.enter_context(tc.tile_pool(name="hpool", bufs=2))
        h16p = stack.enter_context(tc.tile_pool(name="h16p", bufs=2))
        qkp = stack.enter_context(tc.tile_pool(name="qkp", bufs=1))
        vap = stack.enter_context(tc.tile_pool(name="vap", bufs=1))
        p4p = stack.enter_context(tc.tile_pool(name="p4p", bufs=2))
        ekbp = stack.enter_context(tc.tile_pool(name="ekbp", bufs=3))
        up = stack.enter_context(tc.tile_pool(name="up", bufs=2))
        atp = stack.enter_context(tc.tile_pool(name="atp", bufs=1))
        smp = stack.enter_context(tc.tile_pool(name="smp", bufs=3))
        zp = stack.enter_context(tc.tile_pool(name="zp", bufs=1))
        z2p = stack.enter_context(tc.tile_pool(name="z2p", bufs=2))
        aop = stack.enter_context(tc.tile_pool(name="aop", bufs=1))
        ao16p = stack.enter_context(tc.tile_pool(name="ao16p", bufs=1))
        xcp = stack.enter_context(tc.tile_pool(name="xcp", bufs=2))
        xgp = stack.enter_context(tc.tile_pool(name="xgp", bufs=2))
        wst = stack.enter_context(tc.tile_pool(name="wst", bufs=3))   # [128,KC,128] stream
        wvp = stack.enter_context(tc.tile_pool(name="wvp", bufs=2))   # wva halves
        wgp = stack.enter_context(tc.tile_pool(name="wgp", bufs=3))   # glu [128,KC,256]
        wop = stack.enter_context(tc.tile_pool(name="wop", bufs=3))   # wot [128,768]
        bp = stack.enter_context(tc.tile_pool(name="bp", bufs=2))
        bvp = stack.enter_context(tc.tile_pool(name="bvp", bufs=1))

        # ---- constants ----
        c32_t = const.tile([P, NT + S + 2 + 4 * L * KC], F32)
        nc.sync.dma_start(c32_t[:], c32_d[:])
        mb_t = c32_t[:, 0:NT]
        maskb_t = c32_t[:, NT:NT + S]
        lnp_t = c32_t[:, NT + S + 2:]   # [P, 4*L*KC]: l1g|l1b|l2g|l2b per layer
        ones_row = const.tile([1, P], F32R)
        nc.sync.dma_start(ones_row[:], ones_row_d[:].bitcast(F32R))
        ones_col = const.tile([P, 1], F32R)
        nc.sync.dma_start(ones_col[:], ones_col_d[:].bitcast(F32R))
        ones_s = const.tile([1, S], BF16)
        nc.sync.dma_start(ones_s[:], ones_s_d[:])

        # layer 0 hidden state (bf16: matmul operand + residual stream)
        h16_t = h16p.tile([P, KC, S], BF16, tag="h16")
        nc.sync.dma_start(h16_t[:], h16_d[:].rearrange("(c p) t -> p c t", p=P))

        last_gelu = [None]
        prev_exp = [None]
        for l in range(n_layers):
            ln1g_t = lnp_t[:, (4 * l) * KC:(4 * l + 1) * KC]
            ln1b_t = lnp_t[:, (4 * l + 1) * KC:(4 * l + 2) * KC]
            ln2g_t = lnp_t[:, (4 * l + 2) * KC:(4 * l + 3) * KC]
            ln2b_t = lnp_t[:, (4 * l + 3) * KC:(4 * l + 4) * KC]

            with tc.tile_pool(name="qkv_ps", bufs=2, space="PSUM") as qkv_ps, \
                 tc.tile_pool(name="sc_ps", bufs=4, space="PSUM") as sc_ps, \
                 tc.tile_pool(name="pv_ps", bufs=2, space="PSUM") as pv_ps:
                # ---------- V (token-major, head-slotted + ones col) ----------
                bva_t = bvp.tile([1, NH * 65], BF16, tag="bva")
                nc.sync.dma_start(bva_t[:], wslice(l, O_BVA, NH * 65, (1, NH * 65)))
                bqk_t = bp.tile([1, 2 * KC, P], BF16, tag="bqk")
                nc.sync.dma_start(bqk_t[:],
                                  wslice(l, O_BQK, 2 * KC * P, (1, 2 * KC, P)))
                va_t = vap.tile([P, NT, NH * 65], BF16, tag="va")
                for half in range(2):
                    sl = slice(half * HALF, (half + 1) * HALF)
                    wv = wvp.tile([P, KC, HALF], BF16, tag="wv", name=f"wv{half}")
                    nc.sync.dma_start(
                        wv[:], wslice(l, O_V + half * P * KC * HALF, P * KC * HALF,
                                      (P, KC, HALF)))
                    for jt in range(NT):
                        ps = sc_ps.tile([P, HALF], F32, tag="sc",
                                        name=f"vps{half}_{jt}")
                        for kc in range(KC):
                            nc.tensor.matmul(ps[:], h16_t[:, kc, jt * P:(jt + 1) * P],
                                             wv[:, kc, :], start=(kc == 0), stop=False)
                        nc.tensor.matmul(ps[:], ones_s[:, 0:P],
                                         bva_t[:, sl], start=False, stop=True)
                        nc.scalar.activation(va_t[:, jt, sl], ps[:], AF.Copy)

                # ---------- QK + attention, interleaved per head pair ----------
                qk_t = qkp.tile([P, 2 * KC, S], BF16, tag="qk")
                at16 = atp.tile([P, KC, S], BF16, tag="attnT")

                def flush_tail(pend):
                    """Denominator tail of a finished head (deferred one head
                    so the bc matmul's wait on rec doesn't head-of-line-block
                    the next head's score matmuls in the PE FIFO)."""
                    ps_pv, h = pend
                    rec = smp.tile([1, S], F32R, tag="sm", name="rec")
                    nc.vector.reciprocal(rec[:], ps_pv[64:65, :])
                    ps_bc = qkv_ps.tile([64, S], F32, tag="qkvps", name="bc")
                    nc.tensor.matmul(ps_bc[:], ones_row[:, 0:64], rec[:],
                                     start=True, stop=True)
                    rb_sb = up.tile([64, S], F32, tag="rb", name="rb_sb")
                    nc.vector.tensor_copy(rb_sb[:], ps_bc[:])
                    nc.vector.tensor_tensor(
                        at16[64 * (h % 2):64 * (h % 2) + 64, h // 2, :],
                        ps_pv[0:64, :], rb_sb[:], ALU.mult)

                pending = None
                for hp in range(KC):
                    for ot in (KC + hp, hp):     # k chunk, then q chunk
                        ps = qkv_ps.tile([P, S], F32, tag="qkvps",
                                         name=f"qk{ot}")
                        w = wst.tile([P, KC, P], BF16, tag="w", name=f"wqk{ot}")
                        nc.sync.dma_start(
                            w[:], wslice(l, O_QKV + ot * P * KC * P, P * KC * P,
                                         (P, KC, P)))
                        for kc in range(KC):
                            nc.tensor.matmul(ps[:], w[:, kc, :], h16_t[:, kc, :],
                                             start=(kc == 0), stop=False)
                        nc.tensor.matmul(ps[:], bqk_t[:, ot, :], ones_s[:],
                                         start=False, stop=True)
                        nc.scalar.activation(qk_t[:, ot, :], ps[:], AF.Copy)
                    for h in (2 * hp, 2 * hp + 1):
                        koff = (DH * h) % P
                        qoff = (DH * h) % P
                        ekb_t = ekbp.tile([P, NT * S], BF16, tag="ekb",
                                          name=f"ekb{h}")
                        nc.sync.dma_start(ekb_t[:], ekb_d[l, h])
                        p4 = p4p.tile([P, NT, S], BF16, tag="p4")
                        ps_pv = pv_ps.tile([65, S], F32, tag="pv")
                        ps_ss = []
                        for jt in range(NT):
                            ps_s = sc_ps.tile([P, S], F32, tag="sc", name=f"sc{jt}")
                            ps_ss.append(ps_s)
                            nc.tensor.matmul(
                                ps_s[:],
                                qk_t[koff:koff + DH, KC + hp, jt * P:(jt + 1) * P],
                                qk_t[qoff:qoff + DH, hp, :],
                                start=True, stop=True)
                        if pending is not None:
                            flush_tail(pending)
                        for jt in range(NT):
                            _i = nc.scalar.activation(p4[:, jt, :], ps_ss[jt][:],
                                                      AF.Exp,
                                                      bias=mb_t[:, jt:jt + 1],
                                                      scale=1.0)
                            if h == 0 and jt == 0 and last_gelu[0] is not None:
                                add_dep_helper(_i.ins, last_gelu[0].ins, False,
                                               "act table grouping")
                            prev_exp[0] = _i
                            nc.vector.tensor_tensor(
                                p4[:, jt, :], p4[:, jt, :],
                                ekb_t[:, jt * S:(jt + 1) * S], ALU.mult)
                            nc.tensor.matmul(ps_pv[:],
                                             va_t[:, jt, 65 * h:65 * h + 65],
                                             p4[:, jt, :], start=(jt == 0),
                                             stop=(jt == NT - 1))
                        pending = (ps_pv, h)
                flush_tail(pending)
                pending = None

                # ---------- attention out projection + residual ----------
                boa_t = bp.tile([1, KC, P], BF16, tag="boa")
                nc.sync.dma_start(boa_t[:], wslice(l, O_BOA, KC * P, (1, KC, P)))
                z_t = zp.tile([P, KC, S], F32R, tag="z")
                for ot in range(KC):
                    ps = sc_ps.tile([P, S], F32, tag="sc", name=f"prj{ot}")
                    w = wst.tile([P, KC, P], BF16, tag="w", name=f"woa{ot}")
                    nc.sync.dma_start(
                        w[:], wslice(l, O_OA + ot * P * KC * P, P * KC * P,
                                     (P, KC, P)))
                    for kc in range(KC):
                        nc.tensor.matmul(ps[:], w[:, kc, :], at16[:, kc, :],
                                         start=(kc == 0), stop=False)
                    nc.tensor.matmul(ps[:], boa_t[:, ot, :], ones_s[:],
                                     start=False, stop=True)
                    nc.vector.tensor_tensor(z_t[:, ot, :], ps[:],
                                            h16_t[:, ot, :], ALU.add)

            # ---------- LN1 ----------
            ao16 = ao16p.tile([P, KC, S], BF16, tag="ao16")
            _layernorm(nc, tc, z_t, ao16, ln1g_t, ln1b_t, ones_col,
                       ones_row, z2p, smp)

            # ---------- GLU + wo (fused) ----------
            with tc.tile_pool(name="glu_ps", bufs=1, space="PSUM") as glu_ps, \
                 tc.tile_pool(name="wo_ps", bufs=6, space="PSUM") as wo_ps:
                bwo_t = bp.tile([1, KC, P], BF16, tag="bwo")
                nc.sync.dma_start(bwo_t[:], wslice(l, O_BWO, KC * P, (1, KC, P)))

                wo_acc = [wo_ps.tile([P, S], F32, tag="woacc", name=f"woacc{i}")
                          for i in range(KC)]
                for gt in range(NIC):
                    ps_g = glu_ps.tile([P, S], F32, tag="gps")
                    ps_u = glu_ps.tile([P, S], F32, tag="ups")
                    gw = wgp.tile([P, KC, 256], BF16, tag="gw", name=f"gw{gt}")
                    nc.sync.dma_start(
                        gw[:], wslice(l, O_GLU + gt * P * KC * 256, P * KC * 256,
                                      (P, KC, 256)))
                    for kc in range(KC):
                        nc.tensor.matmul(ps_g[:], gw[:, kc, 0:128], ao16[:, kc, :],
                                         start=(kc == 0), stop=(kc == KC - 1))
                    for kc in range(KC):
                        nc.tensor.matmul(ps_u[:], gw[:, kc, 128:256], ao16[:, kc, :],
                                         start=(kc == 0), stop=(kc == KC - 1))
                    xg = xgp.tile([P, S], BF16, tag="xg")
                    _i = nc.scalar.activation(xg[:], ps_g[:], AF.Gelu)
                    if gt == 0 and prev_exp[0] is not None:
                        add_dep_helper(_i.ins, prev_exp[0].ins, False,
                                       "act table grouping")
                    last_gelu[0] = _i
                    xc = xcp.tile([P, S], BF16, tag="xc")
                    nc.vector.tensor_tensor(xc[:], xg[:], ps_u[:], ALU.mult)
                    wot_t = wop.tile([P, HID], BF16, tag="wot")
                    nc.sync.dma_start(
                        wot_t[:], wslice(l, O_OT + gt * P * HID, P * HID, (P, HID)))
                    for ot in range(KC):
                        nc.tensor.matmul(wo_acc[ot][:], wot_t[:, ot * P:(ot + 1) * P],
                                         xc[:], start=(gt == 0), stop=False)
                z2_t = zp.tile([P, KC, S], F32R, tag="z", name="z_mlp")
                for ot in range(KC):
                    nc.tensor.matmul(wo_acc[ot][:], bwo_t[:, ot, :], ones_s[:],
                                     start=False, stop=True)
                    nc.vector.tensor_tensor(z2_t[:, ot, :], wo_acc[ot][:],
                                            ao16[:, ot, :], ALU.add)

            # ---------- LN2 -> next h ----------
            if l + 1 < n_layers:
                h16_t = h16p.tile([P, KC, S], BF16, tag="h16",
                                  name=f"h16_{l + 1}")
                _layernorm(nc, tc, z2_t, h16_t, ln2g_t, ln2b_t, ones_col,
                           ones_row, z2p, smp, ones_s, last_gelu[0])
            else:
                h_t = hpool.tile([P, KC, S], F32R, tag="h", name="h_last")
                _layernorm(nc, tc, z2_t, h_t, ln2g_t, ln2b_t, ones_col,
                           ones_row, z2p, smp, ones_s, last_gelu[0])

        # ---------- final mask + store ----------
        out_sb = zp.tile([P, KC, S], F32, tag="z", name="out_sb")
        if n_layers == 0:
            h_t = hpool.tile([P, KC, S], F32R, tag="h", name="h_last")
            nc.sync.dma_start(h_t[:], hT_d[:].rearrange("(c p) t -> p c t",
                                                        p=P).bitcast(F32R))
        for c in range(KC):
            nc.vector.tensor_tensor(out_sb[:, c, :], h_t[:, c, :].bitcast(F32),
                                    maskb_t[:], ALU.mult)
        nc.sync.dma_start(out_d[:].rearrange("(c p) t -> p c t", p=P), out_sb[:])

        stack.close()
        lp.__exit__(None, None, None)

    nc.finalize()
    return nc


def _prep_inputs(hidden_states, attention_mask, Wqkv_w, Wqkv_b, attn_out_w,
                 attn_out_b, ln1_g, ln1_b, glu_w, wo_w, wo_b, ln2_g, ln2_b,
                 r1, r2, r3):
    """Host-side sharding + weight layout transforms (shared across cores)."""
    f32 = np.float32
    shared = {}
    shared["ones_row"] = np.ones((1, P), f32)
    shared["ones_col"] = np.ones((P, 1), f32)
    shared["ones_s"] = np.ones((1, S), NPBF16)

    # ekb: exp(kerple bias) per (layer, head), Toeplitz [S, S] -> [P, NT*S]
    c1 = np.clip(r1.reshape(L, NH).astype(np.float64), 1e-7, None)
    c2 = np.clip(r2.reshape(L, NH).astype(np.float64), 1e-7, None)
    c3 = np.clip(r3.reshape(L, NH).astype(np.float64), 1e-7, None)
    idx = np.arange(S)
    rel = np.abs(idx[None, :] - idx[:, None]).astype(np.float64)  # [j, i]
    ekb = np.empty((L, NH, P, NT * S), NPBF16)
    for l in range(L):
        for h in range(NH):
            relp = np.where(rel > 0, rel, 1.0) ** c3[l, h]
            relp = np.where(rel > 0, relp, 0.0)
            m = np.exp(-c1[l, h] * np.log1p(c2[l, h] * relp))  # [j, i]
            # [j, i] -> [jt, p, i] -> [p, jt, i] -> [p, jt*i]
            ekb[l, h] = np.ascontiguousarray(
                m.reshape(NT, P, S).transpose(1, 0, 2).reshape(P, NT * S)
            ).astype(NPBF16)
    shared["ekb"] = ekb

    wq = Wqkv_w[:, :HID, :] / 8.0           # fold 1/sqrt(DH)
    wk = Wqkv_w[:, HID:2 * HID, :]
    bq = Wqkv_b[:, :HID] / 8.0
    bk = Wqkv_b[:, HID:2 * HID]
    wqk = np.concatenate([wq, wk], axis=1)  # [L, 1536, HID]
    wqkT = np.transpose(wqk, (0, 2, 1))     # [L, HID, 1536]
    wqk_p = np.ascontiguousarray(
        wqkT.reshape(L, KC, P, 2 * KC, P).transpose(0, 3, 2, 1, 4))
    bqk_p = np.concatenate([bq, bk], axis=1)  # [L, 1536]

    wv = Wqkv_w[:, 2 * HID:, :]             # [L, 768v, 768]
    bv = Wqkv_b[:, 2 * HID:]
    wva = np.zeros((L, HID, NH * 65), f32)
    bva_p = np.zeros((L, NH * 65), f32)
    for h in range(NH):
        wva[:, :, 65 * h:65 * h + 64] = np.transpose(
            wv[:, DH * h:DH * (h + 1), :], (0, 2, 1))
        bva_p[:, 65 * h:65 * h + 64] = bv[:, DH * h:DH * (h + 1)]
        bva_p[:, 65 * h + 64] = 1.0
    wva_p = np.ascontiguousarray(
        wva.reshape(L, KC, P, 2, HALF).transpose(0, 3, 2, 1, 4))

    woaT = np.transpose(attn_out_w, (0, 2, 1))  # [L, HID, HID]
    woa_p = np.ascontiguousarray(
        woaT.reshape(L, KC, P, KC, P).transpose(0, 3, 2, 1, 4))

    glu = np.empty((L, HID, NIC, 256), f32)
    gw = np.transpose(glu_w, (0, 2, 1))     # [L, HID, 6144]
    for gt in range(NIC):
        glu[:, :, gt, 0:128] = gw[:, :, gt * P:(gt + 1) * P]
        glu[:, :, gt, 128:256] = gw[:, :, INTER + gt * P:INTER + (gt + 1) * P]
    glu_p = np.ascontiguousarray(
        glu.reshape(L, KC, P, NIC, 256).transpose(0, 3, 2, 1, 4))
    wot_p = np.ascontiguousarray(np.transpose(wo_w, (0, 2, 1)))  # [L, INTER, HID]

    w16 = np.concatenate([
        wqk_p.reshape(L, -1), wva_p.reshape(L, -1), woa_p.reshape(L, -1),
        glu_p.reshape(L, -1), wot_p.reshape(L, -1),
        bqk_p.reshape(L, -1), bva_p.reshape(L, -1),
        attn_out_b.reshape(L, -1), wo_b.reshape(L, -1),
    ], axis=1).astype(NPBF16)
    shared["w16"] = np.ascontiguousarray(w16.reshape(-1))

    def pcol(v):  # [L, 768] -> [L, P, KC]
        return np.ascontiguousarray(v.reshape(L, KC, P).transpose(0, 2, 1)).astype(f32)

    lnp = np.stack([pcol(ln1_g), pcol(ln1_b), pcol(ln2_g), pcol(ln2_b)],
                   axis=1)  # [L, 4, P, KC]
    lnp = np.ascontiguousarray(lnp.transpose(2, 0, 1, 3)).reshape(P, 4 * L * KC)

    in_maps = []
    for b in range(B):
        m = dict(shared)
        hTb = np.ascontiguousarray(hidden_states[b].T).astype(f32)
        m["hT"] = hTb
        m["h16"] = hTb.astype(NPBF16)
        mask = attention_mask[b].astype(f32)          # [S]
        mbias = (1.0 - mask) * -10000.0
        c32 = np.zeros((P, NT + S + 2 + 4 * L * KC), f32)
        c32[:, 0:NT] = mbias.reshape(NT, P).T
        c32[:, NT:NT + S] = mask[None, :]
        c32[:, NT + S + 2:] = lnp
        m["c32"] = c32
        in_maps.append(m)
    return in_maps


def kernel(**inputs) -> np.ndarray:
    n_layers = int(inputs.pop("_n_layers", L))
    if n_layers not in _BUILT:
        _BUILT[n_layers] = _build(n_layers)
    nc = _BUILT[n_layers]
    in_maps = _prep_inputs(**inputs)
    res = run_bass_kernel_spmd(nc, in_maps, list(range(B))).results
    out = np.empty((B, S, HID), np.float32)
    for b in range(B):
        out[b] = res[b]["out"].T
    return out


# revision 45
# speedup vs baseline: 4.4165x; 1.0903x over previous
"""Bass/Tile TRN2 kernel for nn_BertEncoder_41592463294989.

4-layer BERT encoder, KERPLE attention bias, GLU MLP.
Sharding: data-parallel over batch (B=8 -> 8 cores, 1 sequence each).

Per-core layout: activations transposed [feature, token] so every matmul
contracts over the partition dim and LayerNorm reductions (over features)
are done with ones-vector matmuls on the PE.

v2 design:
 - KERPLE bias is Toeplitz (depends only on |i-j|): exp(bias) is
   precomputed on the HOST per (layer, head) and shipped as a bf16 DRAM
   table; softmax becomes p = exp(s + padmask)*ekb. This removes all
   per-element exp/ln/pow work for the bias on the device (was 2/3 of
   ACT + half of attention DVE time).
 - All matmul operands (weights and activations) are bf16: full PE rate,
   half the weight-DMA bytes, 2x DVE rate on bf16 elementwise ops. The
   residual stream (z, h, ao) stays fp32; bf16 shadow copies feed matmuls.
 - All linear-layer biases are folded into the PE via rank-1 matmuls
   (bias row stationary, ones vector moving) instead of DVE/ACT adds.
 - V weights host-packed into per-head 65-column slots (64 features + a
   ones column) so each PV matmul also produces the softmax denominator.
 - partition broadcasts (1/denominator, LN mu/rstd) via K=1 ones-matmuls.
 - GLU and the wo projection are fused per 128-row chunk.
 - Weights packed into few DRAM tensors (dispatch cost scales with arg
   count in the PJRT path).
"""
import contextlib

import numpy as np
import ml_dtypes

import concourse.bass as bass
from concourse import bacc
import concourse.mybir as mybir
import concourse.tile as tile
from concourse.bass_utils import run_bass_kernel_spmd
from concourse.tile_rust import add_dep_helper

B, S, HID, NH, INTER, L = 8, 512, 768, 12, 3072, 4
DH = HID // NH          # 64
P = 128
NT = S // P             # 4 token tiles
KC = HID // P           # 6 hidden chunks
NIC = INTER // P        # 24 intermediate chunks
F32 = mybir.dt.float32
F32R = mybir.dt.float32r
BF16 = mybir.dt.bfloat16
NPBF16 = ml_dtypes.bfloat16
AF = mybir.ActivationFunctionType
ALU = mybir.AluOpType
HALF = NH * 65 // 2     # 390

_BUILT = {}


def _prefer_combined_act_table(arch):
    """Steer the act-table-load pass to the natural_log_exp set for exp/ln.

    The placement pass greedily first-matches each activation function
    against the table list, so alternating exp/ln picks two different
    tables and reloads on every switch. Removing exp/ln from the
    single-function sets (in the cached dict, canonical indices unchanged)
    makes both resolve to the combined set -> no reloads. The emitted
    act_func_set_id still indexes the canonical act_info.json, and the
    combined table genuinely contains both functions.
    """
    from concourse.hw_specs import get_activation_tables
    tabs = get_activation_tables(arch)
    for nm in list(tabs):
        if nm == "natural_log_exp_and_others":
            continue
        tabs[nm].discard(AF.Exp)
        tabs[nm].discard(AF.Ln)


def _layernorm(nc, tc, z_t, out_t, g_t, b_t, ones_col, ones_row, z2p, smp,
               ones_s=None, act_dep=None):
    """LN over the feature (partition x chunk) axis of z_t [P, KC, S] (F32R).

    out_t may be BF16 (mid-stack: matmul operand + residual) or F32R (last).
    If act_dep is given, a tiny dummy Exp is issued first (ordered after
    act_dep) so the natural_log_exp table load happens off the critical path.
    """
    EPS = 1e-12
    with tc.tile_pool(name="ln_ps", bufs=1, space="PSUM") as ln_ps, \
         tc.tile_pool(name="lnb_ps", bufs=1, space="PSUM") as lnb_ps:
        if act_dep is not None:
            dummy = smp.tile([1, 8], F32, tag="dummy", name="tabswitch")
            _d = nc.scalar.activation(dummy[:], ones_s[:, 0:8], AF.Exp,
                                      bias=0.0, scale=1.0)
            add_dep_helper(_d.ins, act_dep.ins, False, "act table prefetch")
        ps_sz = ln_ps.tile([1, S], F32, tag="sz")
        ps_sz2 = ln_ps.tile([1, S], F32, tag="sz2")
        for c in range(KC):
            nc.tensor.matmul(ps_sz[:], ones_col[:], z_t[:, c, :],
                             start=(c == 0), stop=(c == KC - 1))
        for c in range(KC):
            z2 = z2p.tile([P, S], F32R, tag="ztmp", name=f"zsq{c}")
            nc.scalar.activation(z2[:], z_t[:, c, :].bitcast(F32), AF.Square,
                                 bias=0.0, scale=1.0)
            nc.tensor.matmul(ps_sz2[:], ones_col[:], z2[:],
                             start=(c == 0), stop=(c == KC - 1))
        mu = smp.tile([1, S], F32R, tag="sm", name="mu")
        nc.vector.tensor_scalar(mu[:], ps_sz[:], 1.0 / HID, None, ALU.mult)
        m2 = smp.tile([1, S], F32, tag="sm", name="m2")
        nc.vector.tensor_scalar(m2[:], ps_sz2[:], 1.0 / HID, EPS, ALU.mult, ALU.add)
        musq = smp.tile([1, S], F32, tag="sm", name="musq")
        nc.scalar.activation(musq[:], mu[:].bitcast(F32), AF.Square,
                             bias=0.0, scale=1.0)
        var = smp.tile([1, S], F32, tag="sm", name="var")
        nc.vector.tensor_tensor(var[:], m2[:], musq[:], ALU.subtract)
        lnv = smp.tile([1, S], F32, tag="sm", name="lnv")
        nc.scalar.activation(lnv[:], var[:], AF.Ln, bias=0.0, scale=1.0)
        rstd = smp.tile([1, S], F32R, tag="sm", name="rstd")
        nc.scalar.activation(rstd[:], lnv[:], AF.Exp, bias=0.0, scale=-0.5)
        ps_mu = lnb_ps.tile([P, S], F32, tag="mub")
        nc.tensor.matmul(ps_mu[:], ones_row[:], mu[:], start=True, stop=True)
        ps_rs = lnb_ps.tile([P, S], F32, tag="rsb")
        nc.tensor.matmul(ps_rs[:], ones_row[:], rstd[:], start=True, stop=True)
        for c in range(KC):
            t1 = z2p.tile([P, S], F32, tag="ztmp", name=f"lnt{c}")
            nc.vector.tensor_tensor(t1[:], z_t[:, c, :].bitcast(F32), ps_mu[:],
                                    ALU.subtract)
            nc.vector.tensor_tensor(t1[:], t1[:], ps_rs[:], ALU.mult)
            nc.vector.tensor_scalar(out_t[:, c, :], t1[:], g_t[:, c:c + 1],
                                    b_t[:, c:c + 1], ALU.mult, ALU.add)


def _build(n_layers: int):
    nc = bacc.Bacc("TRN2", target_bir_lowering=False)
    try:
        _prefer_combined_act_table(nc.m.arch)
    except Exception:
        pass

    def inp(name, shape, dt=F32):
        return nc.declare_dram_parameter(name, list(shape), dt, isOutput=False)

    # fp32 consts: mb | maskb | ones_col col | ln params | ones_row row |
    # (n_layers==0 only: hT appended as KC*S columns)
    C32_W = NT + S + 1 + 4 * L * KC + P + (KC * S if n_layers == 0 else 0)
    c32_d = inp("c32", [P, C32_W])
    # bf16: big weight blob, per-layer layout (offsets in elements):
    #   wqk [2KC, P, KC, P] | wva [2, P, KC, HALF] | woa [KC, P, KC, P]
    #   glu [NIC, P, KC, 256] | wot [INTER, HID]
    #   bqk [2KC*P] | bva [NH*65] | boa [KC*P] | bwo [KC*P]
    # then: ones_s [S] | ekb [L, NH, P, NT*S]
    W_QKV = 2 * KC * P * KC * P
    W_V = 2 * P * KC * HALF
    W_OA = KC * P * KC * P
    W_GLU = NIC * P * KC * 256
    W_OT = INTER * HID
    W_B = 2 * KC * P + NH * 65 + KC * P + KC * P
    WLAY = W_QKV + W_V + W_OA + W_GLU + W_OT + W_B
    O_ONES_S = L * WLAY
    O_EKB = O_ONES_S + S
    w16_d = inp("w16", [O_EKB + L * NH * P * NT * S], BF16)
    h16_d = inp("h16", [HID, S], BF16)
    out_d = nc.declare_dram_parameter("out", [HID, S], F32, isOutput=True)

    def wslice(l, off, sz, shape):
        base = l * WLAY + off
        pat = " ".join(f"d{i}" for i in range(len(shape)))
        return w16_d[base:base + sz].rearrange(
            f"({pat}) -> {pat}", **{f"d{i}": shape[i] for i in range(len(shape))})

    O_QKV = 0
    O_V = O_QKV + W_QKV
    O_OA = O_V + W_V
    O_GLU = O_OA + W_OA
    O_OT = O_GLU + W_GLU
    O_BQK = O_OT + W_OT
    O_BVA = O_BQK + 2 * KC * P
    O_BOA = O_BVA + NH * 65
    O_BWO = O_BOA + KC * P

    with tile.TileContext(nc) as tc:
        lp = nc.allow_low_precision(reason="bf16 matmul operands; loose tol")
        lp.__enter__()
        stack = contextlib.ExitStack()
        const = stack.enter_context(tc.tile_pool(name="const", bufs=1))
        hpool = stack.enter_context(tc.tile_pool(name="hpool", bufs=2))
        h16p = stack.enter_context(tc.tile_pool(name="h16p", bufs=2))
        qkp = stack.enter_context(tc.tile_pool(name="qkp", bufs=1))
        vap = stack.enter_context(tc.tile_pool(name="vap", bufs=1))
        p4p = stack.enter_context(tc.tile_pool(name="p4p", bufs=2))
        ekbp = stack.enter_context(tc.tile_pool(name="ekbp", bufs=3))
        up = stack.enter_context(tc.tile_pool(name="up", bufs=2))
        atp = stack.enter_context(tc.tile_pool(name="atp", bufs=1))
        smp = stack.enter_context(tc.tile_pool(name="smp", bufs=3))
        zp = stack.enter_context(tc.tile_pool(name="zp", bufs=1))
        z2p = stack.enter_context(tc.tile_pool(name="z2p", bufs=2))
        aop = stack.enter_context(tc.tile_pool(name="aop", bufs=1))
        ao16p = stack.enter_context(tc.tile_pool(name="ao16p", bufs=1))
        xcp = stack.enter_context(tc.tile_pool(name="xcp", bufs=2))
        xgp = stack.enter_context(tc.tile_pool(name="xgp", bufs=2))
        wst = stack.enter_context(tc.tile_pool(name="wst", bufs=3))   # [128,KC,128] stream
        wvp = stack.enter_context(tc.tile_pool(name="wvp", bufs=2))   # wva halves
        wgp = stack.enter_context(tc.tile_pool(name="wgp", bufs=3))   # glu [128,KC,256]
        wop = stack.enter_context(tc.tile_pool(name="wop", bufs=3))   # wot [128,768]
        bp = stack.enter_context(tc.tile_pool(name="bp", bufs=2))
        bvp = stack.enter_context(tc.tile_pool(name="bvp", bufs=1))

        # ---- constants ----
        c32_t = const.tile([P, NT + S + 1 + 4 * L * KC], F32)
        nc.sync.dma_start(c32_t[:], c32_d[:, 0:NT + S + 1 + 4 * L * KC])
        mb_t = c32_t[:, 0:NT]
        maskb_t = c32_t[:, NT:NT + S]
        ones_col = const.tile([P, 1], F32R)
        nc.sync.dma_start(ones_col[:],
                          c32_d[:, NT + S:NT + S + 1].bitcast(F32R))
        lnp_t = c32_t[:, NT + S + 1:]   # [P, 4*L*KC]: l1g|l1b|l2g|l2b per layer
        ones_row = const.tile([1, P], F32R)
        nc.sync.dma_start(
            ones_row[:],
            c32_d[0:1, NT + S + 1 + 4 * L * KC:
                  NT + S + 1 + 4 * L * KC + P].bitcast(F32R))
        ones_s = const.tile([1, S], BF16)
        nc.sync.dma_start(ones_s[:], w16_d[O_ONES_S:O_ONES_S + S].rearrange(
            "(a b) -> a b", a=1, b=S))

        # layer 0 hidden state (bf16: matmul operand + residual stream)
        h16_t = h16p.tile([P, KC, S], BF16, tag="h16")
        nc.sync.dma_start(h16_t[:], h16_d[:].rearrange("(c p) t -> p c t", p=P))

        last_gelu = [None]
        prev_exp = [None]
        for l in range(n_layers):
            ln1g_t = lnp_t[:, (4 * l) * KC:(4 * l + 1) * KC]
            ln1b_t = lnp_t[:, (4 * l + 1) * KC:(4 * l + 2) * KC]
            ln2g_t = lnp_t[:, (4 * l + 2) * KC:(4 * l + 3) * KC]
            ln2b_t = lnp_t[:, (4 * l + 3) * KC:(4 * l + 4) * KC]

            with tc.tile_pool(name="qkv_ps", bufs=2, space="PSUM") as qkv_ps, \
                 tc.tile_pool(name="sc_ps", bufs=4, space="PSUM") as sc_ps, \
                 tc.tile_pool(name="pv_ps", bufs=2, space="PSUM") as pv_ps:
                # ---------- V (token-major, head-slotted + ones col) ----------
                bva_t = bvp.tile([1, NH * 65], BF16, tag="bva")
                nc.sync.dma_start(bva_t[:], wslice(l, O_BVA, NH * 65, (1, NH * 65)))
                bqk_t = bp.tile([1, 2 * KC, P], BF16, tag="bqk")
                nc.sync.dma_start(bqk_t[:],
                                  wslice(l, O_BQK, 2 * KC * P, (1, 2 * KC, P)))
                va_t = vap.tile([P, NT, NH * 65], BF16, tag="va")
                for half in range(2):
                    sl = slice(half * HALF, (half + 1) * HALF)
                    wv = wvp.tile([P, KC, HALF], BF16, tag="wv", name=f"wv{half}")
                    nc.sync.dma_start(
                        wv[:], wslice(l, O_V + half * P * KC * HALF, P * KC * HALF,
                                      (P, KC, HALF)))
                    for jt in range(NT):
                        ps = sc_ps.tile([P, HALF], F32, tag="sc",
                                        name=f"vps{half}_{jt}")
                        for kc in range(KC):
                            nc.tensor.matmul(ps[:], h16_t[:, kc, jt * P:(jt + 1) * P],
                                             wv[:, kc, :], start=(kc == 0), stop=False)
                        nc.tensor.matmul(ps[:], ones_s[:, 0:P],
                                         bva_t[:, sl], start=False, stop=True)
                        nc.scalar.activation(va_t[:, jt, sl], ps[:], AF.Copy)

                # ---------- QK + attention, interleaved per head pair ----------
                qk_t = qkp.tile([P, 2 * KC, S], BF16, tag="qk")
                at16 = atp.tile([P, KC, S], BF16, tag="attnT")

                def flush_tail(pend):
                    """Denominator tail of a finished head (deferred one head
                    so the bc matmul's wait on rec doesn't head-of-line-block
                    the next head's score matmuls in the PE FIFO)."""
                    ps_pv, h = pend
                    rec = smp.tile([1, S], F32R, tag="sm", name="rec")
                    nc.vector.reciprocal(rec[:], ps_pv[64:65, :])
                    ps_bc = qkv_ps.tile([64, S], F32, tag="qkvps", name="bc")
                    nc.tensor.matmul(ps_bc[:], ones_row[:, 0:64], rec[:],
                                     start=True, stop=True)
                    rb_sb = up.tile([64, S], F32, tag="rb", name="rb_sb")
                    nc.vector.tensor_copy(rb_sb[:], ps_bc[:])
                    nc.vector.tensor_tensor(
                        at16[64 * (h % 2):64 * (h % 2) + 64, h // 2, :],
                        ps_pv[0:64, :], rb_sb[:], ALU.mult)

                pending = None
                for hp in range(KC):
                    for ot in (KC + hp, hp):     # k chunk, then q chunk
                        ps = qkv_ps.tile([P, S], F32, tag="qkvps",
                                         name=f"qk{ot}")
                        w = wst.tile([P, KC, P], BF16, tag="w", name=f"wqk{ot}")
                        nc.sync.dma_start(
                            w[:], wslice(l, O_QKV + ot * P * KC * P, P * KC * P,
                                         (P, KC, P)))
                        for kc in range(KC):
                            nc.tensor.matmul(ps[:], w[:, kc, :], h16_t[:, kc, :],
                                             start=(kc == 0), stop=False)
                        nc.tensor.matmul(ps[:], bqk_t[:, ot, :], ones_s[:],
                                         start=False, stop=True)
                        nc.scalar.activation(qk_t[:, ot, :], ps[:], AF.Copy)
                    for h in (2 * hp, 2 * hp + 1):
                        koff = (DH * h) % P
                        qoff = (DH * h) % P
                        ekb_t = ekbp.tile([P, NT * S], BF16, tag="ekb",
                                          name=f"ekb{h}")
                        eoff = O_EKB + (l * NH + h) * P * NT * S
                        nc.sync.dma_start(
                            ekb_t[:], w16_d[eoff:eoff + P * NT * S].rearrange(
                                "(p x) -> p x", p=P, x=NT * S))
                        p4 = p4p.tile([P, NT, S], BF16, tag="p4")
                        ps_pv = pv_ps.tile([65, S], F32, tag="pv")
                        ps_ss = []
                        for jt in range(NT):
                            ps_s = sc_ps.tile([P, S], F32, tag="sc", name=f"sc{jt}")
                            ps_ss.append(ps_s)
                            nc.tensor.matmul(
                                ps_s[:],
                                qk_t[koff:koff + DH, KC + hp, jt * P:(jt + 1) * P],
                                qk_t[qoff:qoff + DH, hp, :],
                                start=True, stop=True)
                        if pending is not None:
                            flush_tail(pending)
                        for jt in range(NT):
                            _i = nc.scalar.activation(p4[:, jt, :], ps_ss[jt][:],
                                                      AF.Exp,
                                                      bias=mb_t[:, jt:jt + 1],
                                                      scale=1.0)
                            if h == 0 and jt == 0 and last_gelu[0] is not None:
                                add_dep_helper(_i.ins, last_gelu[0].ins, False,
                                               "act table grouping")
                            prev_exp[0] = _i
                            nc.vector.tensor_tensor(
                                p4[:, jt, :], p4[:, jt, :],
                                ekb_t[:, jt * S:(jt + 1) * S], ALU.mult)
                            nc.tensor.matmul(ps_pv[:],
                                             va_t[:, jt, 65 * h:65 * h + 65],
                                             p4[:, jt, :], start=(jt == 0),
                                             stop=(jt == NT - 1))
                        pending = (ps_pv, h)
                flush_tail(pending)
                pending = None

                # ---------- attention out projection + residual ----------
                boa_t = bp.tile([1, KC, P], BF16, tag="boa")
                nc.sync.dma_start(boa_t[:], wslice(l, O_BOA, KC * P, (1, KC, P)))
                z_t = zp.tile([P, KC, S], F32R, tag="z")
                for ot in range(KC):
                    ps = sc_ps.tile([P, S], F32, tag="sc", name=f"prj{ot}")
                    w = wst.tile([P, KC, P], BF16, tag="w", name=f"woa{ot}")
                    nc.sync.dma_start(
                        w[:], wslice(l, O_OA + ot * P * KC * P, P * KC * P,
                                     (P, KC, P)))
                    for kc in range(KC):
                        nc.tensor.matmul(ps[:], w[:, kc, :], at16[:, kc, :],
                                         start=(kc == 0), stop=False)
                    nc.tensor.matmul(ps[:], boa_t[:, ot, :], ones_s[:],
                                     start=False, stop=True)
                    nc.vector.tensor_tensor(z_t[:, ot, :], ps[:],
                                            h16_t[:, ot, :], ALU.add)

            # ---------- LN1 ----------
            ao16 = ao16p.tile([P, KC, S], BF16, tag="ao16")
            _layernorm(nc, tc, z_t, ao16, ln1g_t, ln1b_t, ones_col,
                       ones_row, z2p, smp)

            # ---------- GLU + wo (fused) ----------
            with tc.tile_pool(name="glu_ps", bufs=1, space="PSUM") as glu_ps, \
                 tc.tile_pool(name="wo_ps", bufs=6, space="PSUM") as wo_ps:
                bwo_t = bp.tile([1, KC, P], BF16, tag="bwo")
                nc.sync.dma_start(bwo_t[:], wslice(l, O_BWO, KC * P, (1, KC, P)))

                wo_acc = [wo_ps.tile([P, S], F32, tag="woacc", name=f"woacc{i}")
                          for i in range(KC)]
                for gt in range(NIC):
                    ps_g = glu_ps.tile([P, S], F32, tag="gps")
                    ps_u = glu_ps.tile([P, S], F32, tag="ups")
                    gw = wgp.tile([P, KC, 256], BF16, tag="gw", name=f"gw{gt}")
                    nc.sync.dma_start(
                        gw[:], wslice(l, O_GLU + gt * P * KC * 256, P * KC * 256,
                                      (P, KC, 256)))
                    for kc in range(KC):
                        nc.tensor.matmul(ps_g[:], gw[:, kc, 0:128], ao16[:, kc, :],
                                         start=(kc == 0), stop=(kc == KC - 1))
                    for kc in range(KC):
                        nc.tensor.matmul(ps_u[:], gw[:, kc, 128:256], ao16[:, kc, :],
                                         start=(kc == 0), stop=(kc == KC - 1))
                    xg = xgp.tile([P, S], BF16, tag="xg")
                    _i = nc.scalar.activation(xg[:], ps_g[:], AF.Gelu)
                    if gt == 0 and prev_exp[0] is not None:
                        add_dep_helper(_i.ins, prev_exp[0].ins, False,
                                       "act table grouping")
                    last_gelu[0] = _i
                    xc = xcp.tile([P, S], BF16, tag="xc")
                    nc.vector.tensor_tensor(xc[:], xg[:], ps_u[:], ALU.mult)
                    wot_t = wop.tile([P, HID], BF16, tag="wot")
                    nc.sync.dma_start(
                        wot_t[:], wslice(l, O_OT + gt * P * HID, P * HID, (P, HID)))
                    for ot in range(KC):
                        nc.tensor.matmul(wo_acc[ot][:], wot_t[:, ot * P:(ot + 1) * P],
                                         xc[:], start=(gt == 0), stop=False)
                z2_t = zp.tile([P, KC, S], F32R, tag="z", name="z_mlp")
                for ot in range(KC):
                    nc.tensor.matmul(wo_acc[ot][:], bwo_t[:, ot, :], ones_s[:],
                                     start=False, stop=True)
                    nc.vector.tensor_tensor(z2_t[:, ot, :], wo_acc[ot][:],
                                            ao16[:, ot, :], ALU.add)

            # ---------- LN2 -> next h ----------
            if l + 1 < n_layers:
                h16_t = h16p.tile([P, KC, S], BF16, tag="h16",
                                  name=f"h16_{l + 1}")
                _layernorm(nc, tc, z2_t, h16_t, ln2g_t, ln2b_t, ones_col,
                           ones_row, z2p, smp, ones_s, last_gelu[0])
            else:
                h_t = hpool.tile([P, KC, S], F32R, tag="h", name="h_last")
                _layernorm(nc, tc, z2_t, h_t, ln2g_t, ln2b_t, ones_col,
                           ones_row, z2p, smp, ones_s, last_gelu[0])

        # ---------- final mask + store ----------
        out_sb = zp.tile([P, KC, S], F32, tag="z", name="out_sb")
        if n_layers == 0:
            h_t = hpool.tile([P, KC, S], F32R, tag="h", name="h_last")
            hoff = NT + S + 1 + 4 * L * KC + P
            nc.sync.dma_start(
                h_t[:], c32_d[:, hoff:hoff + KC * S].rearrange(
                    "p (c t) -> p c t", c=KC, t=S).bitcast(F32R))
        for c in range(KC):
            nc.vector.tensor_tensor(out_sb[:, c, :], h_t[:, c, :].bitcast(F32),
                                    maskb_t[:], ALU.mult)
        nc.sync.dma_start(out_d[:].rearrange("(c p) t -> p c t", p=P), out_sb[:])

        stack.close()
        lp.__exit__(None, None, None)

    nc.finalize()
    return nc


def _prep_inputs(hidden_states, attention_mask, Wqkv_w, Wqkv_b, attn_out_w,
                 attn_out_b, ln1_g, ln1_b, glu_w, wo_w, wo_b, ln2_g, ln2_b,
                 r1, r2, r3):
    """Host-side sharding + weight layout transforms (shared across cores)."""
    f32 = np.float32
    shared = {}

    # ekb: exp(kerple bias) per (layer, head), Toeplitz [S, S] -> [P, NT*S]
    c1 = np.clip(r1.reshape(L, NH).astype(np.float64), 1e-7, None)
    c2 = np.clip(r2.reshape(L, NH).astype(np.float64), 1e-7, None)
    c3 = np.clip(r3.reshape(L, NH).astype(np.float64), 1e-7, None)
    idx = np.arange(S)
    rel = np.abs(idx[None, :] - idx[:, None]).astype(np.float64)  # [j, i]
    ekb = np.empty((L, NH, P, NT * S), NPBF16)
    for l in range(L):
        for h in range(NH):
            relp = np.where(rel > 0, rel, 1.0) ** c3[l, h]
            relp = np.where(rel > 0, relp, 0.0)
            m = np.exp(-c1[l, h] * np.log1p(c2[l, h] * relp))  # [j, i]
            # [j, i] -> [jt, p, i] -> [p, jt, i] -> [p, jt*i]
            ekb[l, h] = np.ascontiguousarray(
                m.reshape(NT, P, S).transpose(1, 0, 2).reshape(P, NT * S)
            ).astype(NPBF16)

    wq = Wqkv_w[:, :HID, :] / 8.0           # fold 1/sqrt(DH)
    wk = Wqkv_w[:, HID:2 * HID, :]
    bq = Wqkv_b[:, :HID] / 8.0
    bk = Wqkv_b[:, HID:2 * HID]
    wqk = np.concatenate([wq, wk], axis=1)  # [L, 1536, HID]
    wqkT = np.transpose(wqk, (0, 2, 1))     # [L, HID, 1536]
    wqk_p = np.ascontiguousarray(
        wqkT.reshape(L, KC, P, 2 * KC, P).transpose(0, 3, 2, 1, 4))
    bqk_p = np.concatenate([bq, bk], axis=1)  # [L, 1536]

    wv = Wqkv_w[:, 2 * HID:, :]             # [L, 768v, 768]
    bv = Wqkv_b[:, 2 * HID:]
    wva = np.zeros((L, HID, NH * 65), f32)
    bva_p = np.zeros((L, NH * 65), f32)
    for h in range(NH):
        wva[:, :, 65 * h:65 * h + 64] = np.transpose(
            wv[:, DH * h:DH * (h + 1), :], (0, 2, 1))
        bva_p[:, 65 * h:65 * h + 64] = bv[:, DH * h:DH * (h + 1)]
        bva_p[:, 65 * h + 64] = 1.0
    wva_p = np.ascontiguousarray(
        wva.reshape(L, KC, P, 2, HALF).transpose(0, 3, 2, 1, 4))

    woaT = np.transpose(attn_out_w, (0, 2, 1))  # [L, HID, HID]
    woa_p = np.ascontiguousarray(
        woaT.reshape(L, KC, P, KC, P).transpose(0, 3, 2, 1, 4))

    glu = np.empty((L, HID, NIC, 256), f32)
    gw = np.transpose(glu_w, (0, 2, 1))     # [L, HID, 6144]
    for gt in range(NIC):
        glu[:, :, gt, 0:128] = gw[:, :, gt * P:(gt + 1) * P]
        glu[:, :, gt, 128:256] = gw[:, :, INTER + gt * P:INTER + (gt + 1) * P]
    glu_p = np.ascontiguousarray(
        glu.reshape(L, KC, P, NIC, 256).transpose(0, 3, 2, 1, 4))
    wot_p = np.ascontiguousarray(np.transpose(wo_w, (0, 2, 1)))  # [L, INTER, HID]

    w16 = np.concatenate([
        wqk_p.reshape(L, -1), wva_p.reshape(L, -1), woa_p.reshape(L, -1),
        glu_p.reshape(L, -1), wot_p.reshape(L, -1),
        bqk_p.reshape(L, -1), bva_p.reshape(L, -1),
        attn_out_b.reshape(L, -1), wo_b.reshape(L, -1),
    ], axis=1).astype(NPBF16)
    shared["w16"] = np.ascontiguousarray(np.concatenate([
        w16.reshape(-1), np.ones(S, NPBF16), ekb.reshape(-1)]))

    def pcol(v):  # [L, 768] -> [L, P, KC]
        return np.ascontiguousarray(v.reshape(L, KC, P).transpose(0, 2, 1)).astype(f32)

    lnp = np.stack([pcol(ln1_g), pcol(ln1_b), pcol(ln2_g), pcol(ln2_b)],
                   axis=1)  # [L, 4, P, KC]
    lnp = np.ascontiguousarray(lnp.transpose(2, 0, 1, 3)).reshape(P, 4 * L * KC)

    n_layers = _prep_inputs._n_layers
    in_maps = []
    for b in range(B):
        m = dict(shared)
        hTb = np.ascontiguousarray(hidden_states[b].T).astype(f32)
        m["h16"] = hTb.astype(NPBF16)
        mask = attention_mask[b].astype(f32)          # [S]
        mbias = (1.0 - mask) * -10000.0
        base = NT + S + 1 + 4 * L * KC
        c32 = np.zeros((P, base + P + (KC * S if n_layers == 0 else 0)), f32)
        c32[:, 0:NT] = mbias.reshape(NT, P).T
        c32[:, NT:NT + S] = mask[None, :]
        c32[:, NT + S] = 1.0                      # ones_col
        c32[:, NT + S + 1:base] = lnp
        c32[0, base:base + P] = 1.0               # ones_row
        if n_layers == 0:
            c32[:, base + P:] = hTb.reshape(KC, P, S).transpose(1, 0, 2
                                                               ).reshape(P, KC * S)
        m["c32"] = c32
        in_maps.append(m)
    return in_maps


_prep_inputs._n_layers = L


def kernel(**inputs) -> np.ndarray:
    n_layers = int(inputs.pop("_n_layers", L))
    if n_layers not in _BUILT:
        _BUILT[n_layers] = _build(n_layers)
    nc = _BUILT[n_layers]
    _prep_inputs._n_layers = n_layers
    in_maps = _prep_inputs(**inputs)
    res = run_bass_kernel_spmd(nc, in_maps, list(range(B))).results
    out = np.empty((B, S, HID), np.float32)
    for b in range(B):
        out[b] = res[b]["out"].T
    return out


# revision 50
# speedup vs baseline: 14.8382x; 3.3597x over previous
"""Bass/Tile TRN2 kernel for nn_BertEncoder_41592463294989.

4-layer BERT encoder, KERPLE attention bias, GLU MLP.
Sharding: data-parallel over batch (B=8 -> 8 cores, 1 sequence each).

Per-core layout: activations transposed [feature, token] so every matmul
contracts over the partition dim and LayerNorm reductions (over features)
are done with ones-vector matmuls on the PE.

v2 design:
 - KERPLE bias is Toeplitz (depends only on |i-j|): exp(bias) is
   precomputed on the HOST per (layer, head) and shipped as a bf16 DRAM
   table; softmax becomes p = exp(s + padmask)*ekb. This removes all
   per-element exp/ln/pow work for the bias on the device (was 2/3 of
   ACT + half of attention DVE time).
 - All matmul operands (weights and activations) are bf16: full PE rate,
   half the weight-DMA bytes, 2x DVE rate on bf16 elementwise ops. The
   residual stream (z, h, ao) stays fp32; bf16 shadow copies feed matmuls.
 - All linear-layer biases are folded into the PE via rank-1 matmuls
   (bias row stationary, ones vector moving) instead of DVE/ACT adds.
 - V weights host-packed into per-head 65-column slots (64 features + a
   ones column) so each PV matmul also produces the softmax denominator.
 - partition broadcasts (1/denominator, LN mu/rstd) via K=1 ones-matmuls.
 - GLU and the wo projection are fused per 128-row chunk.
 - Weights packed into few DRAM tensors (dispatch cost scales with arg
   count in the PJRT path).
"""
import contextlib

import numpy as np
import ml_dtypes

import concourse.bass as bass
from concourse import bacc
import concourse.mybir as mybir
import concourse.tile as tile
from concourse.bass_utils import run_bass_kernel_spmd
from concourse.tile_rust import add_dep_helper

B, S, HID, NH, INTER, L = 8, 512, 768, 12, 3072, 4
DH = HID // NH          # 64
P = 128
NT = S // P             # 4 token tiles
KC = HID // P           # 6 hidden chunks
NIC = INTER // P        # 24 intermediate chunks
F32 = mybir.dt.float32
F32R = mybir.dt.float32r
BF16 = mybir.dt.bfloat16
NPBF16 = ml_dtypes.bfloat16
AF = mybir.ActivationFunctionType
ALU = mybir.AluOpType
HALF = NH * 65 // 2     # 390

_BUILT = {}


def _prefer_combined_act_table(arch):
    """Steer the act-table-load pass to the natural_log_exp set for exp/ln.

    The placement pass greedily first-matches each activation function
    against the table list, so alternating exp/ln picks two different
    tables and reloads on every switch. Removing exp/ln from the
    single-function sets (in the cached dict, canonical indices unchanged)
    makes both resolve to the combined set -> no reloads. The emitted
    act_func_set_id still indexes the canonical act_info.json, and the
    combined table genuinely contains both functions.
    """
    from concourse.hw_specs import get_activation_tables
    tabs = get_activation_tables(arch)
    for nm in list(tabs):
        if nm == "natural_log_exp_and_others":
            continue
        tabs[nm].discard(AF.Exp)
        tabs[nm].discard(AF.Ln)


def _layernorm(nc, tc, z_t, out_t, g_t, b_t, ones_col, ones_row, z2p, smp,
               ones_s=None, act_dep=None):
    """LN over the feature (partition x chunk) axis of z_t [P, KC, S] (F32R).

    out_t may be BF16 (mid-stack: matmul operand + residual) or F32R (last).
    If act_dep is given, a tiny dummy Exp is issued first (ordered after
    act_dep) so the natural_log_exp table load happens off the critical path.
    """
    EPS = 1e-12
    with tc.tile_pool(name="ln_ps", bufs=1, space="PSUM") as ln_ps, \
         tc.tile_pool(name="lnb_ps", bufs=1, space="PSUM") as lnb_ps:
        if act_dep is not None:
            dummy = smp.tile([1, 8], F32, tag="dummy", name="tabswitch")
            _d = nc.scalar.activation(dummy[:], ones_s[:, 0:8], AF.Exp,
                                      bias=0.0, scale=1.0)
            add_dep_helper(_d.ins, act_dep.ins, False, "act table prefetch")
        ps_sz = ln_ps.tile([1, S], F32, tag="sz")
        ps_sz2 = ln_ps.tile([1, S], F32, tag="sz2")
        for c in range(KC):
            nc.tensor.matmul(ps_sz[:], ones_col[:], z_t[:, c, :],
                             start=(c == 0), stop=(c == KC - 1))
        for c in range(KC):
            z2 = z2p.tile([P, S], F32R, tag="ztmp", name=f"zsq{c}")
            nc.scalar.activation(z2[:], z_t[:, c, :].bitcast(F32), AF.Square,
                                 bias=0.0, scale=1.0)
            nc.tensor.matmul(ps_sz2[:], ones_col[:], z2[:],
                             start=(c == 0), stop=(c == KC - 1))
        # var*H^2 = H*sz2 - sz^2 (+ eps*H^2); rstd' = rstd/H via Exp(-0.5 ln).
        # The missing 1/H on mu and H on rstd are folded into host-side
        # scaling of the LN gain (g*H) and mu (broadcast of sz/H).
        mu = smp.tile([1, S], F32R, tag="sm", name="mu")
        nc.vector.tensor_scalar(mu[:], ps_sz[:], 1.0 / HID, None, ALU.mult)
        m2h = smp.tile([1, S], F32, tag="sm", name="m2h")
        nc.vector.tensor_scalar(m2h[:], ps_sz2[:], float(HID),
                                EPS * HID * HID, ALU.mult, ALU.add)
        szsq = smp.tile([1, S], F32, tag="sm", name="szsq")
        nc.scalar.activation(szsq[:], ps_sz[:], AF.Square, bias=0.0, scale=1.0)
        var = smp.tile([1, S], F32, tag="sm", name="var")
        nc.vector.tensor_tensor(var[:], m2h[:], szsq[:], ALU.subtract)
        lnv = smp.tile([1, S], F32, tag="sm", name="lnv")
        nc.scalar.activation(lnv[:], var[:], AF.Ln, bias=0.0, scale=1.0)
        rstd = smp.tile([1, S], F32R, tag="sm", name="rstd")
        nc.scalar.activation(rstd[:], lnv[:], AF.Exp, bias=0.0, scale=-0.5)
        ps_mu = lnb_ps.tile([P, S], F32, tag="mub")
        nc.tensor.matmul(ps_mu[:], ones_row[:], mu[:], start=True, stop=True)
        ps_rs = lnb_ps.tile([P, S], F32, tag="rsb")
        nc.tensor.matmul(ps_rs[:], ones_row[:], rstd[:], start=True, stop=True)
        for c in range(KC):
            t1 = z2p.tile([P, S], F32, tag="ztmp", name=f"lnt{c}")
            nc.vector.tensor_tensor(t1[:], z_t[:, c, :].bitcast(F32), ps_mu[:],
                                    ALU.subtract)
            nc.vector.tensor_tensor(t1[:], t1[:], ps_rs[:], ALU.mult)
            nc.vector.tensor_scalar(out_t[:, c, :], t1[:], g_t[:, c:c + 1],
                                    b_t[:, c:c + 1], ALU.mult, ALU.add)


def _build(n_layers: int):
    nc = bacc.Bacc("TRN2", target_bir_lowering=False)
    try:
        _prefer_combined_act_table(nc.m.arch)
    except Exception:
        pass

    def inp(name, shape, dt=F32):
        return nc.declare_dram_parameter(name, list(shape), dt, isOutput=False)

    # fp32 consts: mb | maskb | ones_col col | ln params | ones_row row |
    # (n_layers==0 only: hT appended as KC*S columns)
    C32_W = NT + S + 1 + 4 * L * KC + P + (KC * S if n_layers == 0 else 0)
    c32_d = inp("c32", [P, C32_W])
    # bf16: big weight blob, per-layer layout (offsets in elements):
    #   wqk [2KC, P, KC, P] | wva [2, P, KC, HALF] | woa [KC, P, KC, P]
    #   glu [NIC, P, KC, 256] | wot [INTER, HID]
    #   bqk [2KC*P] | bva [NH*65] | boa [KC*P] | bwo [KC*P]
    # then: ones_s [S] | ekb [L, NH, P, NT*S]
    W_QKV = 2 * KC * P * KC * P
    W_V = 2 * P * KC * HALF
    W_OA = KC * P * KC * P
    W_GLU = NIC * P * KC * 256
    W_OT = INTER * HID
    W_B = 2 * KC * P + NH * 65 + KC * P + KC * P
    WLAY = W_QKV + W_V + W_OA + W_GLU + W_OT + W_B
    O_ONES_S = L * WLAY
    O_EKB = O_ONES_S + S
    w16_d = inp("w16", [O_EKB + L * NH * P * NT * S], BF16)
    h16_d = inp("h16", [HID, S], BF16)
    out_d = nc.declare_dram_parameter("out", [HID, S], F32, isOutput=True)

    def wslice(l, off, sz, shape):
        base = l * WLAY + off
        pat = " ".join(f"d{i}" for i in range(len(shape)))
        return w16_d[base:base + sz].rearrange(
            f"({pat}) -> {pat}", **{f"d{i}": shape[i] for i in range(len(shape))})

    O_QKV = 0
    O_V = O_QKV + W_QKV
    O_OA = O_V + W_V
    O_GLU = O_OA + W_OA
    O_OT = O_GLU + W_GLU
    O_BQK = O_OT + W_OT
    O_BVA = O_BQK + 2 * KC * P
    O_BOA = O_BVA + NH * 65
    O_BWO = O_BOA + KC * P

    with tile.TileContext(nc) as tc:
        lp = nc.allow_low_precision(reason="bf16 matmul operands; loose tol")
        lp.__enter__()
        stack = contextlib.ExitStack()
        const = stack.enter_context(tc.tile_pool(name="const", bufs=1))
        hpool = stack.enter_context(tc.tile_pool(name="hpool", bufs=2))
        h16p = stack.enter_context(tc.tile_pool(name="h16p", bufs=2))
        qkp = stack.enter_context(tc.tile_pool(name="qkp", bufs=1))
        vap = stack.enter_context(tc.tile_pool(name="vap", bufs=1))
        p4p = stack.enter_context(tc.tile_pool(name="p4p", bufs=2))
        ekbp = stack.enter_context(tc.tile_pool(name="ekbp", bufs=3))
        up = stack.enter_context(tc.tile_pool(name="up", bufs=2))
        atp = stack.enter_context(tc.tile_pool(name="atp", bufs=1))
        smp = stack.enter_context(tc.tile_pool(name="smp", bufs=3))
        zp = stack.enter_context(tc.tile_pool(name="zp", bufs=1))
        z2p = stack.enter_context(tc.tile_pool(name="z2p", bufs=2))
        aop = stack.enter_context(tc.tile_pool(name="aop", bufs=1))
        ao16p = stack.enter_context(tc.tile_pool(name="ao16p", bufs=1))
        xcp = stack.enter_context(tc.tile_pool(name="xcp", bufs=2))
        xgp = stack.enter_context(tc.tile_pool(name="xgp", bufs=2))
        wst = stack.enter_context(tc.tile_pool(name="wst", bufs=3))   # [128,KC,128] stream
        wvp = stack.enter_context(tc.tile_pool(name="wvp", bufs=2))   # wva halves
        wgp = stack.enter_context(tc.tile_pool(name="wgp", bufs=3))   # glu [128,KC,256]
        wop = stack.enter_context(tc.tile_pool(name="wop", bufs=3))   # wot [128,768]
        bp = stack.enter_context(tc.tile_pool(name="bp", bufs=2))
        bvp = stack.enter_context(tc.tile_pool(name="bvp", bufs=1))

        # ---- constants (h16 first: layer-0 V matmuls need it soonest) ----
        h16_t = h16p.tile([P, KC, S], BF16, tag="h16")
        nc.sync.dma_start(h16_t[:], h16_d[:].rearrange("(c p) t -> p c t", p=P))
        c32_t = const.tile([P, NT + S + 1 + 4 * L * KC], F32)
        nc.sync.dma_start(c32_t[:], c32_d[:, 0:NT + S + 1 + 4 * L * KC])
        mb_t = c32_t[:, 0:NT]
        maskb_t = c32_t[:, NT:NT + S]
        ones_col = const.tile([P, 1], F32R)
        nc.sync.dma_start(ones_col[:],
                          c32_d[:, NT + S:NT + S + 1].bitcast(F32R))
        lnp_t = c32_t[:, NT + S + 1:]   # [P, 4*L*KC]: l1g|l1b|l2g|l2b per layer
        ones_row = const.tile([1, P], F32R)
        nc.sync.dma_start(
            ones_row[:],
            c32_d[0:1, NT + S + 1 + 4 * L * KC:
                  NT + S + 1 + 4 * L * KC + P].bitcast(F32R))
        ones_s = const.tile([1, S], BF16)
        nc.sync.dma_start(ones_s[:], w16_d[O_ONES_S:O_ONES_S + S].rearrange(
            "(a b) -> a b", a=1, b=S))

        last_gelu = [None]
        prev_exp = [None]
        for l in range(n_layers):
            ln1g_t = lnp_t[:, (4 * l) * KC:(4 * l + 1) * KC]
            ln1b_t = lnp_t[:, (4 * l + 1) * KC:(4 * l + 2) * KC]
            ln2g_t = lnp_t[:, (4 * l + 2) * KC:(4 * l + 3) * KC]
            ln2b_t = lnp_t[:, (4 * l + 3) * KC:(4 * l + 4) * KC]

            with tc.tile_pool(name="qkv_ps", bufs=2, space="PSUM") as qkv_ps, \
                 tc.tile_pool(name="sc_ps", bufs=4, space="PSUM") as sc_ps, \
                 tc.tile_pool(name="pv_ps", bufs=2, space="PSUM") as pv_ps:
                # ---------- V (token-major, head-slotted + ones col) ----------
                bva_t = bvp.tile([1, NH * 65], BF16, tag="bva")
                nc.sync.dma_start(bva_t[:], wslice(l, O_BVA, NH * 65, (1, NH * 65)))
                bqk_t = bp.tile([1, 2 * KC, P], BF16, tag="bqk")
                nc.sync.dma_start(bqk_t[:],
                                  wslice(l, O_BQK, 2 * KC * P, (1, 2 * KC, P)))
                va_t = vap.tile([P, NT, NH * 65], BF16, tag="va")
                for half in range(2):
                    sl = slice(half * HALF, (half + 1) * HALF)
                    wv = wvp.tile([P, KC, HALF], BF16, tag="wv", name=f"wv{half}")
                    nc.sync.dma_start(
                        wv[:], wslice(l, O_V + half * P * KC * HALF, P * KC * HALF,
                                      (P, KC, HALF)))
                    for jt in range(NT):
                        ps = sc_ps.tile([P, HALF], F32, tag="sc",
                                        name=f"vps{half}_{jt}")
                        for kc in range(KC):
                            nc.tensor.matmul(ps[:], h16_t[:, kc, jt * P:(jt + 1) * P],
                                             wv[:, kc, :], start=(kc == 0), stop=False)
                        nc.tensor.matmul(ps[:], ones_s[:, 0:P],
                                         bva_t[:, sl], start=False, stop=True)
                        nc.scalar.activation(va_t[:, jt, sl], ps[:], AF.Copy)

                # ---------- QK + attention, interleaved per head pair ----------
                qk_t = qkp.tile([P, 2 * KC, S], BF16, tag="qk")
                at16 = atp.tile([P, KC, S], BF16, tag="attnT")

                def flush_tail(pend):
                    """Denominator tail of a finished head (deferred one head
                    so the bc matmul's wait on rec doesn't head-of-line-block
                    the next head's score matmuls in the PE FIFO)."""
                    ps_pv, h = pend
                    rec = smp.tile([1, S], F32R, tag="sm", name="rec")
                    nc.vector.reciprocal(rec[:], ps_pv[64:65, :])
                    ps_bc = qkv_ps.tile([64, S], F32, tag="qkvps", name="bc")
                    nc.tensor.matmul(ps_bc[:], ones_row[:, 0:64], rec[:],
                                     start=True, stop=True)
                    rb_sb = up.tile([64, S], F32, tag="rb", name="rb_sb")
                    nc.vector.tensor_copy(rb_sb[:], ps_bc[:])
                    nc.vector.tensor_tensor(
                        at16[64 * (h % 2):64 * (h % 2) + 64, h // 2, :],
                        ps_pv[0:64, :], rb_sb[:], ALU.mult)

                pending = None
                for hp in range(KC):
                    for ot in (KC + hp, hp):     # k chunk, then q chunk
                        ps = qkv_ps.tile([P, S], F32, tag="qkvps",
                                         name=f"qk{ot}")
                        w = wst.tile([P, KC, P], BF16, tag="w", name=f"wqk{ot}")
                        nc.sync.dma_start(
                            w[:], wslice(l, O_QKV + ot * P * KC * P, P * KC * P,
                                         (P, KC, P)))
                        for kc in range(KC):
                            nc.tensor.matmul(ps[:], w[:, kc, :], h16_t[:, kc, :],
                                             start=(kc == 0), stop=False)
                        nc.tensor.matmul(ps[:], bqk_t[:, ot, :], ones_s[:],
                                         start=False, stop=True)
                        nc.scalar.activation(qk_t[:, ot, :], ps[:], AF.Copy)
                    for h in (2 * hp, 2 * hp + 1):
                        koff = (DH * h) % P
                        qoff = (DH * h) % P
                        ekb_t = ekbp.tile([P, NT * S], BF16, tag="ekb",
                                          name=f"ekb{h}")
                        eoff = O_EKB + (l * NH + h) * P * NT * S
                        nc.sync.dma_start(
                            ekb_t[:], w16_d[eoff:eoff + P * NT * S].rearrange(
                                "(p x) -> p x", p=P, x=NT * S))
                        p4 = p4p.tile([P, NT, S], BF16, tag="p4")
                        ps_pv = pv_ps.tile([65, S], F32, tag="pv")
                        ps_ss = []
                        for jt in range(NT):
                            ps_s = sc_ps.tile([P, S], F32, tag="sc", name=f"sc{jt}")
                            ps_ss.append(ps_s)
                            nc.tensor.matmul(
                                ps_s[:],
                                qk_t[koff:koff + DH, KC + hp, jt * P:(jt + 1) * P],
                                qk_t[qoff:qoff + DH, hp, :],
                                start=True, stop=True)
                        if pending is not None:
                            flush_tail(pending)
                        for jt in range(NT):
                            _i = nc.scalar.activation(p4[:, jt, :], ps_ss[jt][:],
                                                      AF.Exp,
                                                      bias=mb_t[:, jt:jt + 1],
                                                      scale=1.0)
                            if h == 0 and jt == 0 and last_gelu[0] is not None:
                                add_dep_helper(_i.ins, last_gelu[0].ins, False,
                                               "act table grouping")
                            prev_exp[0] = _i
                            nc.vector.tensor_tensor(
                                p4[:, jt, :], p4[:, jt, :],
                                ekb_t[:, jt * S:(jt + 1) * S], ALU.mult)
                            nc.tensor.matmul(ps_pv[:],
                                             va_t[:, jt, 65 * h:65 * h + 65],
                                             p4[:, jt, :], start=(jt == 0),
                                             stop=(jt == NT - 1))
                        pending = (ps_pv, h)
                flush_tail(pending)
                pending = None

                # ---------- attention out projection + residual ----------
                boa_t = bp.tile([1, KC, P], BF16, tag="boa")
                nc.sync.dma_start(boa_t[:], wslice(l, O_BOA, KC * P, (1, KC, P)))
                z_t = zp.tile([P, KC, S], F32R, tag="z")
                for ot in range(KC):
                    ps = sc_ps.tile([P, S], F32, tag="sc", name=f"prj{ot}")
                    w = wst.tile([P, KC, P], BF16, tag="w", name=f"woa{ot}")
                    nc.sync.dma_start(
                        w[:], wslice(l, O_OA + ot * P * KC * P, P * KC * P,
                                     (P, KC, P)))
                    for kc in range(KC):
                        nc.tensor.matmul(ps[:], w[:, kc, :], at16[:, kc, :],
                                         start=(kc == 0), stop=False)
                    nc.tensor.matmul(ps[:], boa_t[:, ot, :], ones_s[:],
                                     start=False, stop=True)
                    nc.vector.tensor_tensor(z_t[:, ot, :], ps[:],
                                            h16_t[:, ot, :], ALU.add)

            # ---------- LN1 ----------
            ao16 = ao16p.tile([P, KC, S], BF16, tag="ao16")
            _layernorm(nc, tc, z_t, ao16, ln1g_t, ln1b_t, ones_col,
                       ones_row, z2p, smp)

            # ---------- GLU + wo (fused) ----------
            with tc.tile_pool(name="glu_ps", bufs=1, space="PSUM") as glu_ps, \
                 tc.tile_pool(name="wo_ps", bufs=6, space="PSUM") as wo_ps:
                bwo_t = bp.tile([1, KC, P], BF16, tag="bwo")
                nc.sync.dma_start(bwo_t[:], wslice(l, O_BWO, KC * P, (1, KC, P)))

                wo_acc = [wo_ps.tile([P, S], F32, tag="woacc", name=f"woacc{i}")
                          for i in range(KC)]
                for gt in range(NIC):
                    ps_g = glu_ps.tile([P, S], F32, tag="gps")
                    ps_u = glu_ps.tile([P, S], F32, tag="ups")
                    gw = wgp.tile([P, KC, 256], BF16, tag="gw", name=f"gw{gt}")
                    nc.sync.dma_start(
                        gw[:], wslice(l, O_GLU + gt * P * KC * 256, P * KC * 256,
                                      (P, KC, 256)))
                    for kc in range(KC):
                        nc.tensor.matmul(ps_g[:], gw[:, kc, 0:128], ao16[:, kc, :],
                                         start=(kc == 0), stop=(kc == KC - 1))
                    for kc in range(KC):
                        nc.tensor.matmul(ps_u[:], gw[:, kc, 128:256], ao16[:, kc, :],
                                         start=(kc == 0), stop=(kc == KC - 1))
                    xg = xgp.tile([P, S], BF16, tag="xg")
                    _i = nc.scalar.activation(xg[:], ps_g[:], AF.Gelu)
                    if gt == 0 and prev_exp[0] is not None:
                        add_dep_helper(_i.ins, prev_exp[0].ins, False,
                                       "act table grouping")
                    last_gelu[0] = _i
                    xc = xcp.tile([P, S], BF16, tag="xc")
                    nc.vector.tensor_tensor(xc[:], xg[:], ps_u[:], ALU.mult)
                    wot_t = wop.tile([P, HID], BF16, tag="wot")
                    nc.sync.dma_start(
                        wot_t[:], wslice(l, O_OT + gt * P * HID, P * HID, (P, HID)))
                    for ot in range(KC):
                        nc.tensor.matmul(wo_acc[ot][:], wot_t[:, ot * P:(ot + 1) * P],
                                         xc[:], start=(gt == 0), stop=False)
                z2_t = zp.tile([P, KC, S], F32R, tag="z", name="z_mlp")
                for ot in range(KC):
                    nc.tensor.matmul(wo_acc[ot][:], bwo_t[:, ot, :], ones_s[:],
                                     start=False, stop=True)
                    nc.vector.tensor_tensor(z2_t[:, ot, :], wo_acc[ot][:],
                                            ao16[:, ot, :], ALU.add)

            # ---------- LN2 -> next h ----------
            if l + 1 < n_layers:
                h16_t = h16p.tile([P, KC, S], BF16, tag="h16",
                                  name=f"h16_{l + 1}")
                _layernorm(nc, tc, z2_t, h16_t, ln2g_t, ln2b_t, ones_col,
                           ones_row, z2p, smp, ones_s, last_gelu[0])
            else:
                h_t = hpool.tile([P, KC, S], F32R, tag="h", name="h_last")
                _layernorm(nc, tc, z2_t, h_t, ln2g_t, ln2b_t, ones_col,
                           ones_row, z2p, smp, ones_s, last_gelu[0])

        # ---------- final mask + store ----------
        out_sb = zp.tile([P, KC, S], F32, tag="z", name="out_sb")
        if n_layers == 0:
            h_t = hpool.tile([P, KC, S], F32R, tag="h", name="h_last")
            hoff = NT + S + 1 + 4 * L * KC + P
            nc.sync.dma_start(
                h_t[:], c32_d[:, hoff:hoff + KC * S].rearrange(
                    "p (c t) -> p c t", c=KC, t=S).bitcast(F32R))
        out_view = out_d[:].rearrange("(c p) t -> p c t", p=P)
        for c in range(KC):
            nc.vector.tensor_tensor(out_sb[:, c, :], h_t[:, c, :].bitcast(F32),
                                    maskb_t[:], ALU.mult)
            nc.sync.dma_start(out_view[:, c, :], out_sb[:, c, :])

        stack.close()
        lp.__exit__(None, None, None)

    nc.finalize()
    return nc


def _prep_inputs(hidden_states, attention_mask, Wqkv_w, Wqkv_b, attn_out_w,
                 attn_out_b, ln1_g, ln1_b, glu_w, wo_w, wo_b, ln2_g, ln2_b,
                 r1, r2, r3):
    """Host-side sharding + weight layout transforms (shared across cores)."""
    f32 = np.float32
    shared = {}

    # ekb: exp(kerple bias) per (layer, head), Toeplitz [S, S] -> [P, NT*S]
    c1 = np.clip(r1.reshape(L, NH).astype(np.float64), 1e-7, None)
    c2 = np.clip(r2.reshape(L, NH).astype(np.float64), 1e-7, None)
    c3 = np.clip(r3.reshape(L, NH).astype(np.float64), 1e-7, None)
    idx = np.arange(S)
    rel = np.abs(idx[None, :] - idx[:, None]).astype(np.float64)  # [j, i]
    ekb = np.empty((L, NH, P, NT * S), NPBF16)
    for l in range(L):
        for h in range(NH):
            relp = np.where(rel > 0, rel, 1.0) ** c3[l, h]
            relp = np.where(rel > 0, relp, 0.0)
            m = np.exp(-c1[l, h] * np.log1p(c2[l, h] * relp))  # [j, i]
            # [j, i] -> [jt, p, i] -> [p, jt, i] -> [p, jt*i]
            ekb[l, h] = np.ascontiguousarray(
                m.reshape(NT, P, S).transpose(1, 0, 2).reshape(P, NT * S)
            ).astype(NPBF16)

    wq = Wqkv_w[:, :HID, :] / 8.0           # fold 1/sqrt(DH)
    wk = Wqkv_w[:, HID:2 * HID, :]
    bq = Wqkv_b[:, :HID] / 8.0
    bk = Wqkv_b[:, HID:2 * HID]
    wqk = np.concatenate([wq, wk], axis=1)  # [L, 1536, HID]
    wqkT = np.transpose(wqk, (0, 2, 1))     # [L, HID, 1536]
    wqk_p = np.ascontiguousarray(
        wqkT.reshape(L, KC, P, 2 * KC, P).transpose(0, 3, 2, 1, 4))
    bqk_p = np.concatenate([bq, bk], axis=1)  # [L, 1536]

    wv = Wqkv_w[:, 2 * HID:, :]             # [L, 768v, 768]
    bv = Wqkv_b[:, 2 * HID:]
    wva = np.zeros((L, HID, NH * 65), f32)
    bva_p = np.zeros((L, NH * 65), f32)
    for h in range(NH):
        wva[:, :, 65 * h:65 * h + 64] = np.transpose(
            wv[:, DH * h:DH * (h + 1), :], (0, 2, 1))
        bva_p[:, 65 * h:65 * h + 64] = bv[:, DH * h:DH * (h + 1)]
        bva_p[:, 65 * h + 64] = 1.0
    wva_p = np.ascontiguousarray(
        wva.reshape(L, KC, P, 2, HALF).transpose(0, 3, 2, 1, 4))

    woaT = np.transpose(attn_out_w, (0, 2, 1))  # [L, HID, HID]
    woa_p = np.ascontiguousarray(
        woaT.reshape(L, KC, P, KC, P).transpose(0, 3, 2, 1, 4))

    glu = np.empty((L, HID, NIC, 256), f32)
    gw = np.transpose(glu_w, (0, 2, 1))     # [L, HID, 6144]
    for gt in range(NIC):
        glu[:, :, gt, 0:128] = gw[:, :, gt * P:(gt + 1) * P]
        glu[:, :, gt, 128:256] = gw[:, :, INTER + gt * P:INTER + (gt + 1) * P]
    glu_p = np.ascontiguousarray(
        glu.reshape(L, KC, P, NIC, 256).transpose(0, 3, 2, 1, 4))
    wot_p = np.ascontiguousarray(np.transpose(wo_w, (0, 2, 1)))  # [L, INTER, HID]

    w16 = np.concatenate([
        wqk_p.reshape(L, -1), wva_p.reshape(L, -1), woa_p.reshape(L, -1),
        glu_p.reshape(L, -1), wot_p.reshape(L, -1),
        bqk_p.reshape(L, -1), bva_p.reshape(L, -1),
        attn_out_b.reshape(L, -1), wo_b.reshape(L, -1),
    ], axis=1).astype(NPBF16)
    shared["w16"] = np.ascontiguousarray(np.concatenate([
        w16.reshape(-1), np.ones(S, NPBF16), ekb.reshape(-1)]))

    def pcol(v):  # [L, 768] -> [L, P, KC]
        return np.ascontiguousarray(v.reshape(L, KC, P).transpose(0, 2, 1)).astype(f32)

    # gains scaled by HID: the kernel computes rstd/HID (see _layernorm)
    lnp = np.stack([pcol(ln1_g * HID), pcol(ln1_b), pcol(ln2_g * HID),
                    pcol(ln2_b)], axis=1)  # [L, 4, P, KC]
    lnp = np.ascontiguousarray(lnp.transpose(2, 0, 1, 3)).reshape(P, 4 * L * KC)

    n_layers = _prep_inputs._n_layers
    in_maps = []
    for b in range(B):
        m = dict(shared)
        hTb = np.ascontiguousarray(hidden_states[b].T).astype(f32)
        m["h16"] = hTb.astype(NPBF16)
        mask = attention_mask[b].astype(f32)          # [S]
        mbias = (1.0 - mask) * -10000.0
        base = NT + S + 1 + 4 * L * KC
        c32 = np.zeros((P, base + P + (KC * S if n_layers == 0 else 0)), f32)
        c32[:, 0:NT] = mbias.reshape(NT, P).T
        c32[:, NT:NT + S] = mask[None, :]
        c32[:, NT + S] = 1.0                      # ones_col
        c32[:, NT + S + 1:base] = lnp
        c32[0, base:base + P] = 1.0               # ones_row
        if n_layers == 0:
            c32[:, base + P:] = hTb.reshape(KC, P, S).transpose(1, 0, 2
                                                               ).reshape(P, KC * S)
        m["c32"] = c32
        in_maps.append(m)
    return in_maps


_prep_inputs._n_layers = L


def kernel(**inputs) -> np.ndarray:
    n_layers = int(inputs.pop("_n_layers", L))
    if n_layers not in _BUILT:
        _BUILT[n_layers] = _build(n_layers)
    nc = _BUILT[n_layers]
    _prep_inputs._n_layers = n_layers
    in_maps = _prep_inputs(**inputs)
    res = run_bass_kernel_spmd(nc, in_maps, list(range(B))).results
    out = np.empty((B, S, HID), np.float32)
    for b in range(B):
        out[b] = res[b]["out"].T
    return out
